# revision 51
# baseline (speedup 1.0000x reference)
# Trainium2 Bass kernel for the 4-branch cross-attention block.
#
# Problem: N=4 batches, L1=L2=1024, D=512, H=8 heads of 64.
#   q1,k1,v1 = proj(input1); q2,k2,v2 = proj(input2)
#   four attention branches (q1k1v1, q1k2v2, q2k1v1, q2k2v2), masked softmax
#   over the key axis, outputs averaged pairwise.
#
# Sharding: 8 cores = 4 batches x 2 head-groups (4 heads each). SPMD — one
# program, per-core data.
#
# Device-side dataflow (per core, 16 branch-heads of L x L attention):
#   ST   = K @ Q^T          (keys on partitions, queries on the free axis;
#                            lhsT = zero-padded kz block, rhs = qT, fp16)
#   P    = exp(ST)          (ACT engine; host pre-zeroed masked tokens in x,
#                            so masked keys give exp(0)=1 against v=0 rows
#                            and a masked ones-column — they drop out of both
#                            the numerator and the denominator exactly)
#   O^T  = [V | m]^T @ P    (bf16; mask column yields denominators in row 64)
#   r    = approx_recip(denom)   (DVE; query-mask and the 0.5 average factor
#                                 are applied on the host after gather)
#   rbc  = partition_broadcast(r)  (gpsimd)
#   out += O^T * rbc        (DVE, bf16 accumulator)
#
# Schedule (the exp stream on ACT, 128 tiles of [128,1024] @ ~1.03us = 132us,
# is the pipeline floor; PE total is within ~5% of it, so the projections
# must ride inside the stream's slack, not ahead of it):
#   - every input is host-packed so its DMA moves its full per-partition
#     span contiguously (DMA time is descriptor-rate bound, so fewer/bigger
#     descriptors win);
#   - launches go out in parallel on the SP/ACT/gpsimd queues, priority
#     ordered: the x1 halves ride two queues, wk1/wq1(ot0) right behind;
#   - the kz zero-fills run on gpsimd — anywhere else they head-of-line
#     block the projection casts (DVE and ACT queues are strictly in-order);
#   - the prefix projects k1/q1(ot0) + v1(lt0,lt1); the remaining 22 pieces
#     are spliced into branches 0-5 in PAIRS after kt 2/4/6 so the st-pool's
#     2-slot rotation parity (and with it the QK one-tile lookahead ahead of
#     ACT) survives;
#   - each branch's normalize chain is deferred one branch and its combine
#     two branches, so every DVE op is data-ready when the in-order DVE
#     queue reaches it — a waiting DVE op would stall the projection casts
#     and with them the QK slot rotation that feeds ACT;
#   - the last branch's kt7 exp + PV are split into column halves so its
#     normalize chain overlaps the final exps instead of trailing them.

import sys

sys.path.insert(0, "/opt/trn_rl_repo")

import ml_dtypes
import numpy as np

import concourse.bacc as bacc
import concourse.mybir as mybir
import concourse.tile as tile
from concourse.bass_utils import run_bass_kernel_spmd

F32 = mybir.dt.float32
F16 = mybir.dt.float16
BF16 = mybir.dt.bfloat16
EXP = mybir.ActivationFunctionType.Exp

L = 1024  # sequence length (both sides)
D = 512  # hidden
NB = 4  # batches
HPG = 4  # heads per core (head group)
HD = 64  # head size
OG = HPG * HD  # output channels per core = 256
KT = L // 128  # 8 key tiles
DT = D // 128  # 4 contraction tiles for projections

_NC = None  # cached compiled program
TRACE = False  # set by test harness to capture an NTFF profile
LAST_RESULT = None  # full BassKernelResults of the last run (for profiling)


def _tt(pool, shape, dtype, tag):
    return pool.tile(shape, dtype, tag=tag, name=tag)


def _install_ntff_hook():
    # antenv.axon_hooks is absent in this image; provide it so
    # run_bass_kernel_spmd(trace=True) can capture NTFF profiles.
    import types, contextlib, ctypes

    if "antenv.axon_hooks" in sys.modules:
        return
    lib = ctypes.CDLL("/opt/axon/libaxon_pjrt.so")
    lib.axon_start_nrt_profile.argtypes = [
        ctypes.POINTER(ctypes.c_int64),
        ctypes.c_size_t,
    ]
    lib.axon_start_nrt_profile.restype = ctypes.c_int64
    lib.axon_stop_nrt_profile.argtypes = [ctypes.c_char_p]
    lib.axon_stop_nrt_profile.restype = ctypes.c_int64

    @contextlib.contextmanager
    def _hook(output_dir, device_ids):
        import jax

        jax.devices()
        if device_ids:
            ids = (ctypes.c_int64 * len(device_ids))(*device_ids)
            rc = lib.axon_start_nrt_profile(ids, len(device_ids))
        else:
            rc = lib.axon_start_nrt_profile(None, 0)
        if rc != 0:
            raise RuntimeError(f"axon_start_nrt_profile rc={rc}")
        try:
            yield
        finally:
            n = lib.axon_stop_nrt_profile(str(output_dir).encode())
            print(f"ntff profile: {n} file(s) in {output_dir}", file=sys.stderr)

    mod = types.ModuleType("antenv.axon_hooks")
    mod.get_axon_ntff_profile_hook = lambda: _hook
    mod.set_axon_ntff_profile_hook = lambda h: None
    sys.modules["antenv.axon_hooks"] = mod


def _build():
    nc = bacc.Bacc("TRN2", target_bir_lowering=False, debug=False, num_devices=8)

    # x and weights arrive as fp16 (host-converted): halves input DMA and
    # makes every projection LDWEIGHTS a cheap 2-byte load, at ~8x finer
    # quantization than bf16 (which overshoots the 2e-2 error budget).
    # side 1 arrives as two packed tensors [x-half | critical ot0 weights]
    # so the whole branch-0 prefix is gated by exactly two parallel DMAs.
    x1a_d = nc.declare_dram_parameter("x1a", [128, 2 * L + DT * 128], F16,
                                      isOutput=False)
    x1b_d = nc.declare_dram_parameter("x1b", [128, 2 * L + DT * 128], F16,
                                      isOutput=False)
    x_d = {2: nc.declare_dram_parameter("x2T", [128, DT * L], F16,
                                        isOutput=False)}
    ws = {}
    for wn in ("wq1", "wk1", "wq2", "wk2"):
        # [p][ot][dk][c128]
        ws[wn] = nc.declare_dram_parameter(wn, [128, 2 * DT * 128], F16,
                                           isOutput=False)
    for wn in ("wv1", "wv2"):
        # [p][dk][c256]
        ws[wn] = nc.declare_dram_parameter(wn, [128, DT * OG], F16,
                                           isOutput=False)
    m4_d = {s: nc.declare_dram_parameter(f"m4{s}", [128, KT * HPG], F32,
                                         isOutput=False) for s in (1, 2)}
    out_d = {s: nc.declare_dram_parameter(f"out{s}T", [OG, L], BF16, isOutput=True)
             for s in (1, 2)}
    # branches 14/15 ship raw f32 accumulators (numerator + denominator
    # row) instead of normalizing on-device — their normalize chains would
    # otherwise trail the exp stream serially. The host divides.
    oa_d = {bi: nc.declare_dram_parameter(f"oa{bi}", [HD + 1, L], F32,
                                          isOutput=True) for bi in (14, 15)}

    with tile.TileContext(nc) as tc:
        with (
            tc.tile_pool(name="pers", bufs=1) as pers,
            tc.tile_pool(name="pt", bufs=20) as ptp,
            tc.tile_pool(name="sm", bufs=2) as smp,
            tc.tile_pool(name="st", bufs=2, space="PSUM") as stp,
            tc.tile_pool(name="acc", bufs=2, space="PSUM") as accp,
        ):
            # ---- persistent tiles ----
            x_r, w_r, m4_sb, kz, qT = {}, {}, {}, {}, {}
            v_e = {1: [], 2: []}
            x1ab = {0: _tt(pers, [128, 2 * L + DT * 128], F16, "x1a"),
                    1: _tt(pers, [128, 2 * L + DT * 128], F16, "x1b")}
            for s in (1, 2):
                if s == 2:
                    x_r[s] = _tt(pers, [128, DT, L], F16, f"x{s}")
                m4_sb[s] = _tt(pers, [128, KT, HPG], F32, f"m4{s}")
                kz[s] = _tt(pers, [128, HPG * KT * 128], F16, f"kz{s}")
                qT[s] = _tt(pers, [128, 2, L], F16, f"q{s}T")
            for wn in ("wq1", "wk1", "wq2", "wk2"):
                w_r[wn] = _tt(pers, [128, 2, DT, 128], F16, wn)
            for wn in ("wv1", "wv2"):
                w_r[wn] = _tt(pers, [128, DT, OG], F16, wn)
            outacc = {qs: _tt(pers, [HD, HPG, L], BF16, f"out{qs}")
                      for qs in (1, 2)}

            junk = _tt(pers, [128, 512], F16, "junk")
            nc.vector.memset(junk[:], 1.0)
            junk_e = _tt(pers, [128, 512], BF16, "junk_e")

            # ---- input DMA: parallel queues, priority order ----
            # HWDGE queues are SP + ACT only; gpsimd launches via SWDGE and
            # also hosts the kz zero-fills (it is otherwise idle early).
            def _w_dma(eng, wn, ot):
                eng.dma_start(
                    w_r[wn][:, ot, :, :],
                    ws[wn][:, ot * 512:(ot + 1) * 512]
                    .rearrange("p (dk c) -> p dk c", c=128))

            def _w_dma_all(eng, wn):
                eng.dma_start(
                    w_r[wn][:],
                    ws[wn][:].rearrange("p (ot dk c) -> p ot dk c", ot=2,
                                        c=128))

            def _x_dma(eng, s, half):
                assert s == 2
                eng.dma_start(
                    x_r[s][:, 2 * half:2 * half + 2, :],
                    x_d[s][:, 2048 * half:2048 * half + 2048]
                    .rearrange("p (dk c) -> p dk c", c=L))

            def _wv_dma(eng, s):
                eng.dma_start(w_r[f"wv{s}"][:],
                              ws[f"wv{s}"][:].rearrange("p (dk c) -> p dk c",
                                                        c=OG))

            # Only the branch-0 critical set launches up front — everything
            # else is deferred into the branch loop so the DMA engines spend
            # the head exclusively on x1/wk1/wq1/wv1. (Each queue serializes
            # its transfers at ~1.7us fixed + bytes at ~140GB/s, and the
            # engine pool is shared across queues.)
            nc.sync.dma_start(x1ab[0][:], x1a_d[:])
            nc.scalar.dma_start(x1ab[1][:], x1b_d[:])
            _wv_dma(nc.gpsimd, 1)
            nc.gpsimd.dma_start(m4_sb[1][:].rearrange("p a b -> p (a b)"),
                                m4_d[1][:])
            # exp-table preload + ACT warm-up while the inputs stream in
            nc.scalar.activation(junk_e[:], junk[:], EXP)
            nc.gpsimd.memset(kz[1][:], 0.0)

            # The scheduler front-loads dependency-free DMA launches, so the
            # non-critical transfers would otherwise compete with x1 for the
            # shared DMA engines. Gate each one on x1's arrival by writing a
            # corner of its destination (WAW dep) sourced from x1b.
            gq = x1ab[1][0:1, 0:1]
            for corner in (
                x_r[2][0:1, 0, 0:1], x_r[2][0:1, 2, 0:1],
                w_r["wk1"][0:1, 1, 0, 0:1], w_r["wq1"][0:1, 1, 0, 0:1],
                w_r["wv2"][0:1, 0, 0:1],
                w_r["wq2"][0:1, 0, 0, 0:1], w_r["wk2"][0:1, 0, 0, 0:1],
            ):
                nc.gpsimd.tensor_copy(corner, gq)
            _w_dma(nc.gpsimd, "wk1", 1)
            _w_dma(nc.gpsimd, "wq1", 1)
            _x_dma(nc.sync, 2, 0)
            _x_dma(nc.sync, 2, 1)
            _wv_dma(nc.sync, 2)
            _w_dma_all(nc.sync, "wq2")
            _w_dma_all(nc.sync, "wk2")
            nc.gpsimd.dma_start(m4_sb[2][:].rearrange("p a b -> p (a b)"),
                                m4_d[2][:])
            nc.gpsimd.memset(kz[2][:], 0.0)

            # PE warm-up: the HAM clock gate releases only after ~3.4us of
            # sustained activity; these bridge the DMA window so the first
            # projections run at full rate.
            for _ in range(8):
                wps = _tt(stp, [128, 512], F32, "st")
                nc.tensor.matmul(wps[:], junk[:, 0:128], junk[:], start=True,
                                 stop=True)

            # ---- projection pieces ([128, 512] PSUM chunks) ----
            # qT per side: [128, 2, L] (tile ht holds heads 2ht, 2ht+1).
            # kz per side: [128, HPG*KT*128] zero-padded per (head, kt) block
            # so QK's moving qT streams all 128 partitions at full rate.
            def x_chunk(s, dk, c0, c1):
                # side 1's x lives in the packed x1a/x1b tiles
                if s == 1:
                    t, loc = x1ab[dk // 2], dk % 2
                    return t[:, loc * L + c0:loc * L + c1]
                return x_r[s][:, dk, c0:c1]

            def w_chunk(kind, s, ot, dk):
                if s == 1 and ot == 0:
                    t = x1ab[0] if kind == "q" else x1ab[1]
                    return t[:, 2 * L + dk * 128:2 * L + (dk + 1) * 128]
                return w_r[f"w{kind}{s}"][:, ot, dk, :]

            def k_piece(s, ot, nh, act_h0=False):
                # output chans = heads 2ot,2ot+1 ; keys nh*512:(nh+1)*512
                ps = _tt(stp, [128, 512], F32, "st")
                for dk in range(DT):
                    nc.tensor.matmul(
                        ps[:], w_chunk("k", s, ot, dk),
                        x_chunk(s, dk, nh * 512, (nh + 1) * 512),
                        start=(dk == 0), stop=(dk == DT - 1))
                for hh in range(2):
                    h = 2 * ot + hh
                    po = hh * 64
                    base = h * KT * 128 + nh * 512
                    # the pre-stream head-0 cast can ride the idle ACT engine
                    # to shorten the DVE chain ahead of the first QK
                    eng = nc.scalar.copy if (act_h0 and hh == 0) else \
                        nc.vector.tensor_copy
                    eng(kz[s][po:po + 64, base:base + 512], ps[po:po + 64, :])

            def q_piece(s, ot, nh, act=False):
                ps = _tt(stp, [128, 512], F32, "st")
                for dk in range(DT):
                    nc.tensor.matmul(
                        ps[:], w_chunk("q", s, ot, dk),
                        x_chunk(s, dk, nh * 512, (nh + 1) * 512),
                        start=(dk == 0), stop=(dk == DT - 1))
                eng = nc.scalar.copy if act else nc.vector.tensor_copy
                eng(qT[s][:, ot, nh * 512:(nh + 1) * 512], ps[:])

            # v in natural layout with mask column: [128, HPG, 65] per key tile
            def v_piece(s, lt):
                w = w_r[f"wv{s}"]
                ps = _tt(stp, [128, OG], F32, "st")
                for dk in range(DT):
                    nc.tensor.matmul(
                        ps[:], x_chunk(s, dk, lt * 128, (lt + 1) * 128),
                        w[:, dk, :], start=(dk == 0), stop=(dk == DT - 1))
                t = _tt(pers, [128, HPG, HD + 1], BF16, f"v{s}_{lt}")
                nc.vector.tensor_copy(
                    t[:, :, 0:HD], ps[:].rearrange("p (h d) -> p h d", h=HPG))
                # mask column on gpsimd: these tiny copies would head-of-line
                # block the critical kz/qT casts on the in-order DVE queue.
                nc.gpsimd.tensor_copy(t[:, :, HD:HD + 1],
                                      m4_sb[s][:, lt, :, None])
                assert len(v_e[s]) == lt
                v_e[s].append(t)

            # ---- prefix: just enough projection for branch 0's first QK ----
            # q first: the first QK needs both qT casts but only the first
            # kz cast, so this ordering minimizes the DVE chain ahead of it.
            q_piece(1, 0, 0)
            q_piece(1, 0, 1, act=True)
            k_piece(1, 0, 0, act_h0=True)
            k_piece(1, 0, 1)

            # Remaining pieces, spliced into branches in pairs (pairs keep
            # the 2-slot "st" rotation parity so QK keeps its one-tile
            # lookahead ahead of the exp stream). Branch 0 takes all of v1
            # at kt 1/3/5/7; later branches take pairs at kt 2/4/6.
            splices = {
                0: [[lambda: v_piece(1, 0), lambda: v_piece(1, 1)],
                    [lambda: v_piece(1, 2), lambda: v_piece(1, 3)],
                    [lambda: v_piece(1, 4), lambda: v_piece(1, 5)],
                    [lambda: v_piece(1, 6), lambda: v_piece(1, 7)]],
                1: [[lambda: k_piece(1, 1, 0), lambda: k_piece(1, 1, 1)],
                    [lambda: q_piece(1, 1, 0), lambda: q_piece(1, 1, 1)]],
                2: [[lambda: q_piece(2, 0, 0), lambda: q_piece(2, 0, 1)],
                    [lambda: q_piece(2, 1, 0), lambda: q_piece(2, 1, 1)]],
                3: [[lambda: k_piece(2, 0, 0), lambda: k_piece(2, 0, 1)]],
                4: [[lambda: v_piece(2, 0), lambda: v_piece(2, 1)],
                    [lambda: v_piece(2, 2), lambda: v_piece(2, 3)]],
                5: [[lambda: v_piece(2, 4), lambda: v_piece(2, 5)],
                    [lambda: v_piece(2, 6), lambda: v_piece(2, 7)]],
                # kz2-ot1 feeds branches 10/12+ — park it out in branch 6
                # where the PE has recovered its slack.
                6: [[lambda: k_piece(2, 1, 0), lambda: k_piece(2, 1, 1)]],
            }

            # ---- attention ----
            branches = [(ks, qs, h) for ks in (1, 2) for qs in (1, 2)
                        for h in range(HPG)]

            def emit_qk(ks, qs, h, kt, stage=False):
                st = _tt(stp, [128, L], F32, "st")
                blk = (h * KT + kt) * 128
                for nh in range(2):
                    nc.tensor.matmul(
                        st[:, nh * 512:(nh + 1) * 512],
                        kz[ks][:, blk:blk + 128],
                        qT[qs][:, h // 2, nh * 512:(nh + 1) * 512],
                        start=True,
                        stop=True,
                    )
                if stage:
                    # bounce the logits to SBUF: the exp reads SBUF at the
                    # same ACT cost, but the PSUM st slot frees as soon as
                    # the copy lands — deepening the QK lookahead beyond the
                    # two PSUM slots so projection-splice bursts on the PE
                    # don't starve the exp stream.
                    ss = smp.tile([128, L], F32, tag="sst", name="sst", bufs=3)
                    nc.vector.tensor_copy(ss[:], st[:])
                    return ss
                return st

            def emit_combine(p):
                # two branches deferred: by now the gpsimd broadcast of r is
                # long done, so these DVE ops never block the pipeline.
                ks, qs, h, acc, rbc = p
                oslice = outacc[qs][:, h, :]
                if ks == 1:
                    nc.vector.tensor_mul(oslice, acc[0:HD, :], rbc[:])
                else:
                    tmp = _tt(smp, [64, L], BF16, "tmp")
                    nc.vector.tensor_mul(tmp[:], acc[0:HD, :], rbc[:])
                    nc.vector.tensor_add(oslice, oslice, tmp[:])
                    nc.sync.dma_start(
                        out_d[qs][h * HD:(h + 1) * HD, :], oslice)

            def norm_pend(ks, qs, h, acc):
                # normalization scalar r = 1/denom in [1, L] (query mask and
                # the 0.5 averaging factor are applied on the host),
                # broadcast to 64 partitions on the (otherwise idle) gpsimd.
                s_sb = _tt(smp, [1, L], F32, "s_sb")
                nc.vector.tensor_copy(s_sb[:], acc[HD:HD + 1, :])
                rinv = _tt(smp, [1, L], F32, "rinv")
                nc.vector.reciprocal_approx_fast(rinv[:], s_sb[:])
                rbc = _tt(smp, [64, L], F32, "rbc")
                nc.gpsimd.partition_broadcast(rbc[:], rinv[:])
                return (ks, qs, h, acc, rbc)

            norm_q = []  # accs awaiting normalize (one branch old)
            combine_q = []  # pends awaiting combine (two branches old)
            look_st = None
            for bi, (ks, qs, h) in enumerate(branches):
                last = bi == len(branches) - 1
                spl = splices.get(bi, [])
                # QK for all 8 key tiles first: the PE free-runs one tile
                # ahead of ACT (throttled by the two st PSUM slots).
                sts = [look_st] if look_st is not None else []
                kts = (1, 3, 5, 7) if bi == 0 else (2, 4, 6)
                for kt in range(len(sts), KT):
                    sts.append(emit_qk(ks, qs, h, kt))
                    if kt in kts and spl:
                        for fn in spl.pop(0):
                            fn()
                pts = []
                for kt in range(KT):
                    if last and kt == KT - 1:
                        # split the final exp tile so the raw export of the
                        # first column half overlaps the second half.
                        for nhh in range(2):
                            pt = _tt(ptp, [128, 512], BF16, "pt")
                            nc.scalar.activation(
                                pt[:], sts[kt][:, nhh * 512:(nhh + 1) * 512],
                                EXP)
                            pts.append(pt)
                    else:
                        pt = _tt(ptp, [128, L], BF16, "pt")
                        nc.scalar.activation(pt[:], sts[kt][:], EXP)
                        pts.append(pt)
                # flush: combines of branch i-2, then normalizes of branch
                # i-1 — everything data-ready by now, so the in-order DVE
                # queue never parks on them.
                while combine_q:
                    emit_combine(combine_q.pop(0))
                while norm_q:
                    combine_q.append(norm_pend(*norm_q.pop(0)))
                if bi == 14:
                    # branches 14/15 skip the on-device combine: ship their
                    # ks=1 partners' halves now, raw accumulators later.
                    for hh in (2, 3):
                        nc.sync.dma_start(
                            out_d[2][hh * HD:(hh + 1) * HD, :],
                            outacc[2][:, hh, :])
                acc = _tt(accp, [HD + 1, L], F32, "acc")
                for kt in range(KT):
                    if kt == KT - 1 and not last:
                        # software-pipeline: next branch's first QK goes ahead
                        # of this branch's last PV so ACT rolls over gap-free.
                        look_st = emit_qk(*branches[bi + 1][:3], 0)
                    if last and kt == KT - 1:
                        for nh in range(2):
                            nc.tensor.matmul(
                                acc[:, nh * 512:(nh + 1) * 512],
                                v_e[ks][kt][:, h, :], pts[KT - 1 + nh][:],
                                start=False, stop=True)
                        for nh in range(2):
                            # raw export per column half: one DVE copy + DMA,
                            # no normalize chain in the kernel tail. The two
                            # halves ship on different queues.
                            sl = slice(nh * 512, (nh + 1) * 512)
                            o = _tt(smp, [HD + 1, L], F32, "oacc")
                            nc.vector.tensor_copy(o[:, sl], acc[:, sl])
                            # both halves on sync: a late SWDGE (gpsimd) DMA
                            # would extend the queue-drain in the epilogue.
                            nc.sync.dma_start(oa_d[15][:, sl], o[:, sl])
                    else:
                        for nh in range(2):
                            nc.tensor.matmul(
                                acc[:, nh * 512:(nh + 1) * 512],
                                v_e[ks][kt][:, h, :],
                                pts[kt][:, nh * 512:(nh + 1) * 512],
                                start=(kt == 0),
                                stop=(kt == KT - 1),
                            )
                if bi == 14:
                    o = _tt(smp, [HD + 1, L], F32, "oacc")
                    nc.vector.tensor_copy(o[:], acc[:])
                    nc.sync.dma_start(oa_d[14][:], o[:])
                elif not last:
                    norm_q.append((ks, qs, h, acc))

    nc.compile()
    return nc


def kernel(**inputs):
    global _NC
    if _NC is None:
        _NC = _build()

    mask1 = np.asarray(inputs["mask1"], dtype=np.float32)
    mask2 = np.asarray(inputs["mask2"], dtype=np.float32)
    # pre-zero masked tokens: masked keys then contribute exp(0)*0 = 0 to
    # both the attention numerator and (via the v mask column) denominator.
    x1 = np.asarray(inputs["input1"], dtype=np.float32) * mask1[:, :, None]
    x2 = np.asarray(inputs["input2"], dtype=np.float32) * mask2[:, :, None]
    W = {k: np.asarray(inputs[k], dtype=np.float32) for k in
         ("Wq1", "Wk1", "Wv1", "Wq2", "Wk2", "Wv2")}

    def pack_x(xb):
        # [L, D] -> xT [D, L] -> [p][dk][c] flattened
        return np.ascontiguousarray(
            xb.T.astype(np.float16).reshape(DT, 128, L)
            .transpose(1, 0, 2).reshape(128, DT * L))

    def pack_wqk(w, og):
        # W.T[:, og] is [D, OG] = [dk p][ot c128] -> [p][ot][dk][c]
        return np.ascontiguousarray(
            w.T[:, og].astype(np.float16).reshape(DT, 128, 2, 128)
            .transpose(1, 2, 0, 3).reshape(128, 2 * DT * 128))

    def pack_wv(w, og):
        # [dk p][c] -> [p][dk][c]
        return np.ascontiguousarray(
            w.T[:, og].astype(np.float16).reshape(DT, 128, OG)
            .transpose(1, 0, 2).reshape(128, DT * OG))

    in_maps = []
    for core in range(8):
        b, hg = core // 2, core % 2
        og = slice(hg * OG, (hg + 1) * OG)
        xp1 = pack_x(x1[b])
        wq1p = pack_wqk(W["Wq1"], og)
        wk1p = pack_wqk(W["Wk1"], og)
        m = {
            "x1a": np.ascontiguousarray(
                np.concatenate([xp1[:, :2 * L], wq1p[:, :DT * 128]], axis=1)),
            "x1b": np.ascontiguousarray(
                np.concatenate([xp1[:, 2 * L:], wk1p[:, :DT * 128]], axis=1)),
            "x2T": pack_x(x2[b]),
            "m41": np.ascontiguousarray(
                np.repeat(mask1[b].reshape(KT, 128).T[:, :, None], HPG, axis=2)
                .reshape(128, KT * HPG)),
            "m42": np.ascontiguousarray(
                np.repeat(mask2[b].reshape(KT, 128).T[:, :, None], HPG, axis=2)
                .reshape(128, KT * HPG)),
        }
        for s in (1, 2):
            m[f"wq{s}"] = pack_wqk(W[f"Wq{s}"], og)
            m[f"wk{s}"] = pack_wqk(W[f"Wk{s}"], og)
            m[f"wv{s}"] = pack_wv(W[f"Wv{s}"], og)
        in_maps.append(m)

    global LAST_RESULT
    if TRACE:
        _install_ntff_hook()
    res = run_bass_kernel_spmd(_NC, in_maps, list(range(8)), trace=TRACE)
    LAST_RESULT = res

    # query-side mask + the 0.5 branch-average factor are applied here (the
    # device ships out = sum_branches O/denom, unmasked).
    hm1 = (0.5 * mask1)[:, :, None]
    hm2 = (0.5 * mask2)[:, :, None]
    output1 = np.empty((NB, L, D), dtype=np.float32)
    output2 = np.empty((NB, L, D), dtype=np.float32)
    for core in range(8):
        b, hg = core // 2, core % 2
        og = slice(hg * OG, (hg + 1) * OG)
        output1[b, :, og] = np.asarray(res.results[core]["out1T"],
                                       dtype=np.float32).T
        o2 = np.asarray(res.results[core]["out2T"], dtype=np.float32)
        # branches 14/15 = (ks=2, qs=2, h=2/3) shipped raw accumulators;
        # divide and add their terms to the ks=1 halves in out2T.
        for bi, hh in ((14, 2), (15, 3)):
            a = np.asarray(res.results[core][f"oa{bi}"], dtype=np.float32)
            o2[hh * HD:(hh + 1) * HD, :] += a[:HD, :] / a[HD:HD + 1, :]
        output2[b, :, og] = o2.T
    output1 *= hm1
    output2 *= hm2
    return (output1, output2)


# revision 52
# speedup vs baseline: 1.0029x; 1.0029x over previous
# Trainium2 Bass kernel for the 4-branch cross-attention block.
#
# Problem: N=4 batches, L1=L2=1024, D=512, H=8 heads of 64.
#   q1,k1,v1 = proj(input1); q2,k2,v2 = proj(input2)
#   four attention branches (q1k1v1, q1k2v2, q2k1v1, q2k2v2), masked softmax
#   over the key axis, outputs averaged pairwise.
#
# Sharding: 8 cores = 4 batches x 2 head-groups (4 heads each). SPMD — one
# program, per-core data.
#
# Device-side dataflow (per core, 16 branch-heads of L x L attention):
#   ST   = K @ Q^T          (keys on partitions, queries on the free axis;
#                            lhsT = zero-padded kz block, rhs = qT, fp16)
#   P    = exp(ST)          (ACT engine; host pre-zeroed masked tokens in x,
#                            so masked keys give exp(0)=1 against v=0 rows
#                            and a masked ones-column — they drop out of both
#                            the numerator and the denominator exactly)
#   O^T  = [V | m]^T @ P    (bf16; mask column yields denominators in row 64)
#   r    = approx_recip(denom)   (DVE; query-mask and the 0.5 average factor
#                                 are applied on the host after gather)
#   rbc  = partition_broadcast(r)  (gpsimd)
#   out += O^T * rbc        (DVE, bf16 accumulator)
#
# Schedule (the exp stream on ACT, 128 tiles of [128,1024] @ ~1.03us = 132us,
# is the pipeline floor; PE total is within ~5% of it, so the projections
# must ride inside the stream's slack, not ahead of it):
#   - every input is host-packed so its DMA moves its full per-partition
#     span contiguously (DMA time is descriptor-rate bound, so fewer/bigger
#     descriptors win);
#   - launches go out in parallel on the SP/ACT/gpsimd queues, priority
#     ordered: the x1 halves ride two queues, wk1/wq1(ot0) right behind;
#   - the kz zero-fills run on gpsimd — anywhere else they head-of-line
#     block the projection casts (DVE and ACT queues are strictly in-order);
#   - the prefix projects k1/q1(ot0) + v1(lt0,lt1); the remaining 22 pieces
#     are spliced into branches 0-5 in PAIRS after kt 2/4/6 so the st-pool's
#     2-slot rotation parity (and with it the QK one-tile lookahead ahead of
#     ACT) survives;
#   - each branch's normalize chain is deferred one branch and its combine
#     two branches, so every DVE op is data-ready when the in-order DVE
#     queue reaches it — a waiting DVE op would stall the projection casts
#     and with them the QK slot rotation that feeds ACT;
#   - the last branch's kt7 exp + PV are split into column halves so its
#     normalize chain overlaps the final exps instead of trailing them.

import sys

sys.path.insert(0, "/opt/trn_rl_repo")

import ml_dtypes
import numpy as np

import concourse.bacc as bacc
import concourse.mybir as mybir
import concourse.tile as tile
from concourse.bass_utils import run_bass_kernel_spmd

F32 = mybir.dt.float32
F16 = mybir.dt.float16
BF16 = mybir.dt.bfloat16
EXP = mybir.ActivationFunctionType.Exp

L = 1024  # sequence length (both sides)
D = 512  # hidden
NB = 4  # batches
HPG = 4  # heads per core (head group)
HD = 64  # head size
OG = HPG * HD  # output channels per core = 256
KT = L // 128  # 8 key tiles
DT = D // 128  # 4 contraction tiles for projections

_NC = None  # cached compiled program
TRACE = False  # set by test harness to capture an NTFF profile
LAST_RESULT = None  # full BassKernelResults of the last run (for profiling)


def _tt(pool, shape, dtype, tag):
    return pool.tile(shape, dtype, tag=tag, name=tag)


def _install_ntff_hook():
    # antenv.axon_hooks is absent in this image; provide it so
    # run_bass_kernel_spmd(trace=True) can capture NTFF profiles.
    import types, contextlib, ctypes

    if "antenv.axon_hooks" in sys.modules:
        return
    lib = ctypes.CDLL("/opt/axon/libaxon_pjrt.so")
    lib.axon_start_nrt_profile.argtypes = [
        ctypes.POINTER(ctypes.c_int64),
        ctypes.c_size_t,
    ]
    lib.axon_start_nrt_profile.restype = ctypes.c_int64
    lib.axon_stop_nrt_profile.argtypes = [ctypes.c_char_p]
    lib.axon_stop_nrt_profile.restype = ctypes.c_int64

    @contextlib.contextmanager
    def _hook(output_dir, device_ids):
        import jax

        jax.devices()
        if device_ids:
            ids = (ctypes.c_int64 * len(device_ids))(*device_ids)
            rc = lib.axon_start_nrt_profile(ids, len(device_ids))
        else:
            rc = lib.axon_start_nrt_profile(None, 0)
        if rc != 0:
            raise RuntimeError(f"axon_start_nrt_profile rc={rc}")
        try:
            yield
        finally:
            n = lib.axon_stop_nrt_profile(str(output_dir).encode())
            print(f"ntff profile: {n} file(s) in {output_dir}", file=sys.stderr)

    mod = types.ModuleType("antenv.axon_hooks")
    mod.get_axon_ntff_profile_hook = lambda: _hook
    mod.set_axon_ntff_profile_hook = lambda h: None
    sys.modules["antenv.axon_hooks"] = mod


def _build():
    nc = bacc.Bacc("TRN2", target_bir_lowering=False, debug=False, num_devices=8)

    # x and weights arrive as fp16 (host-converted): halves input DMA and
    # makes every projection LDWEIGHTS a cheap 2-byte load, at ~8x finer
    # quantization than bf16 (which overshoots the 2e-2 error budget).
    # side 1 arrives as two packed tensors [x-half | critical ot0 weights]
    # so the whole branch-0 prefix is gated by exactly two parallel DMAs.
    x1a_d = nc.declare_dram_parameter("x1a", [128, 2 * L + DT * 128], F16,
                                      isOutput=False)
    x1b_d = nc.declare_dram_parameter("x1b", [128, 2 * L + DT * 128], F16,
                                      isOutput=False)
    x_d = {2: nc.declare_dram_parameter("x2T", [128, DT * L], F16,
                                        isOutput=False)}
    ws = {}
    for wn in ("wq1", "wk1", "wq2", "wk2"):
        # [p][ot][dk][c128]
        ws[wn] = nc.declare_dram_parameter(wn, [128, 2 * DT * 128], F16,
                                           isOutput=False)
    for wn in ("wv1", "wv2"):
        # [p][dk][c256]
        ws[wn] = nc.declare_dram_parameter(wn, [128, DT * OG], F16,
                                           isOutput=False)
    m4_d = {s: nc.declare_dram_parameter(f"m4{s}", [128, KT * HPG], F32,
                                         isOutput=False) for s in (1, 2)}
    out_d = {s: nc.declare_dram_parameter(f"out{s}T", [OG, L], BF16, isOutput=True)
             for s in (1, 2)}
    # branches 14/15 ship raw f32 accumulators (numerator + denominator
    # row) instead of normalizing on-device — their normalize chains would
    # otherwise trail the exp stream serially. The host divides.
    oa_d = {bi: nc.declare_dram_parameter(f"oa{bi}", [HD + 1, L], F32,
                                          isOutput=True) for bi in (14, 15)}

    with tile.TileContext(nc) as tc:
        with (
            tc.tile_pool(name="pers", bufs=1) as pers,
            tc.tile_pool(name="pt", bufs=20) as ptp,
            tc.tile_pool(name="sm", bufs=2) as smp,
            tc.tile_pool(name="st", bufs=2, space="PSUM") as stp,
            tc.tile_pool(name="acc", bufs=2, space="PSUM") as accp,
        ):
            # ---- persistent tiles ----
            x_r, w_r, m4_sb, kz, qT = {}, {}, {}, {}, {}
            v_e = {1: [], 2: []}
            x1ab = {0: _tt(pers, [128, 2 * L + DT * 128], F16, "x1a"),
                    1: _tt(pers, [128, 2 * L + DT * 128], F16, "x1b")}
            for s in (1, 2):
                if s == 2:
                    x_r[s] = _tt(pers, [128, DT, L], F16, f"x{s}")
                m4_sb[s] = _tt(pers, [128, KT, HPG], F32, f"m4{s}")
                kz[s] = _tt(pers, [128, HPG * KT * 128], F16, f"kz{s}")
                qT[s] = _tt(pers, [128, 2, L], F16, f"q{s}T")
            for wn in ("wq1", "wk1", "wq2", "wk2"):
                w_r[wn] = _tt(pers, [128, 2, DT, 128], F16, wn)
            for wn in ("wv1", "wv2"):
                w_r[wn] = _tt(pers, [128, DT, OG], F16, wn)
            outacc = {qs: _tt(pers, [HD, HPG, L], BF16, f"out{qs}")
                      for qs in (1, 2)}

            junk = _tt(pers, [128, 512], F16, "junk")
            nc.vector.memset(junk[:], 1.0)
            junk_e = _tt(pers, [128, 512], BF16, "junk_e")

            # ---- input DMA: parallel queues, priority order ----
            # HWDGE queues are SP + ACT only; gpsimd launches via SWDGE and
            # also hosts the kz zero-fills (it is otherwise idle early).
            def _w_dma(eng, wn, ot):
                eng.dma_start(
                    w_r[wn][:, ot, :, :],
                    ws[wn][:, ot * 512:(ot + 1) * 512]
                    .rearrange("p (dk c) -> p dk c", c=128))

            def _w_dma_all(eng, wn):
                eng.dma_start(
                    w_r[wn][:],
                    ws[wn][:].rearrange("p (ot dk c) -> p ot dk c", ot=2,
                                        c=128))

            def _x_dma(eng, s, half):
                assert s == 2
                eng.dma_start(
                    x_r[s][:, 2 * half:2 * half + 2, :],
                    x_d[s][:, 2048 * half:2048 * half + 2048]
                    .rearrange("p (dk c) -> p dk c", c=L))

            def _wv_dma(eng, s):
                eng.dma_start(w_r[f"wv{s}"][:],
                              ws[f"wv{s}"][:].rearrange("p (dk c) -> p dk c",
                                                        c=OG))

            # Only the branch-0 critical set launches up front — everything
            # else is deferred into the branch loop so the DMA engines spend
            # the head exclusively on x1/wk1/wq1/wv1. (Each queue serializes
            # its transfers at ~1.7us fixed + bytes at ~140GB/s, and the
            # engine pool is shared across queues.)
            nc.sync.dma_start(x1ab[0][:], x1a_d[:])
            nc.scalar.dma_start(x1ab[1][:], x1b_d[:])
            _wv_dma(nc.gpsimd, 1)
            nc.gpsimd.dma_start(m4_sb[1][:].rearrange("p a b -> p (a b)"),
                                m4_d[1][:])
            # exp-table preload + ACT warm-up while the inputs stream in
            nc.scalar.activation(junk_e[:], junk[:], EXP)
            nc.gpsimd.memset(kz[1][:], 0.0)

            # The scheduler front-loads dependency-free DMA launches, so the
            # non-critical transfers would otherwise compete with x1 for the
            # shared DMA engines. Gate each one on x1's arrival by writing a
            # corner of its destination (WAW dep) sourced from x1b.
            gq = x1ab[1][0:1, 0:1]
            for corner in (
                x_r[2][0:1, 0, 0:1], x_r[2][0:1, 2, 0:1],
                w_r["wk1"][0:1, 1, 0, 0:1], w_r["wq1"][0:1, 1, 0, 0:1],
                w_r["wv2"][0:1, 0, 0:1],
                w_r["wq2"][0:1, 0, 0, 0:1], w_r["wk2"][0:1, 0, 0, 0:1],
            ):
                nc.gpsimd.tensor_copy(corner, gq)
            _w_dma(nc.gpsimd, "wk1", 1)
            _w_dma(nc.gpsimd, "wq1", 1)
            _x_dma(nc.sync, 2, 0)
            _x_dma(nc.sync, 2, 1)
            _wv_dma(nc.sync, 2)
            _w_dma_all(nc.sync, "wq2")
            _w_dma_all(nc.sync, "wk2")
            nc.gpsimd.dma_start(m4_sb[2][:].rearrange("p a b -> p (a b)"),
                                m4_d[2][:])
            nc.gpsimd.memset(kz[2][:], 0.0)

            # PE warm-up: the HAM clock gate releases only after ~3.4us of
            # sustained activity; these bridge the DMA window so the first
            # projections run at full rate.
            for _ in range(8):
                wps = _tt(stp, [128, 512], F32, "st")
                nc.tensor.matmul(wps[:], junk[:, 0:128], junk[:], start=True,
                                 stop=True)

            # ---- projection pieces ([128, 512] PSUM chunks) ----
            # qT per side: [128, 2, L] (tile ht holds heads 2ht, 2ht+1).
            # kz per side: [128, HPG*KT*128] zero-padded per (head, kt) block
            # so QK's moving qT streams all 128 partitions at full rate.
            def x_chunk(s, dk, c0, c1):
                # side 1's x lives in the packed x1a/x1b tiles
                if s == 1:
                    t, loc = x1ab[dk // 2], dk % 2
                    return t[:, loc * L + c0:loc * L + c1]
                return x_r[s][:, dk, c0:c1]

            def w_chunk(kind, s, ot, dk):
                if s == 1 and ot == 0:
                    t = x1ab[0] if kind == "q" else x1ab[1]
                    return t[:, 2 * L + dk * 128:2 * L + (dk + 1) * 128]
                return w_r[f"w{kind}{s}"][:, ot, dk, :]

            def k_piece(s, ot, nh, act_h0=False):
                # output chans = heads 2ot,2ot+1 ; keys nh*512:(nh+1)*512
                ps = _tt(stp, [128, 512], F32, "st")
                for dk in range(DT):
                    nc.tensor.matmul(
                        ps[:], w_chunk("k", s, ot, dk),
                        x_chunk(s, dk, nh * 512, (nh + 1) * 512),
                        start=(dk == 0), stop=(dk == DT - 1))
                for hh in range(2):
                    h = 2 * ot + hh
                    po = hh * 64
                    base = h * KT * 128 + nh * 512
                    # the pre-stream head-0 cast can ride the idle ACT engine
                    # to shorten the DVE chain ahead of the first QK
                    eng = nc.scalar.copy if (act_h0 and hh == 0) else \
                        nc.vector.tensor_copy
                    eng(kz[s][po:po + 64, base:base + 512], ps[po:po + 64, :])

            def q_piece(s, ot, nh, act=False):
                ps = _tt(stp, [128, 512], F32, "st")
                for dk in range(DT):
                    nc.tensor.matmul(
                        ps[:], w_chunk("q", s, ot, dk),
                        x_chunk(s, dk, nh * 512, (nh + 1) * 512),
                        start=(dk == 0), stop=(dk == DT - 1))
                eng = nc.scalar.copy if act else nc.vector.tensor_copy
                eng(qT[s][:, ot, nh * 512:(nh + 1) * 512], ps[:])

            # v in natural layout with mask column: [128, HPG, 65] per key tile
            def v_piece(s, lt):
                w = w_r[f"wv{s}"]
                ps = _tt(stp, [128, OG], F32, "st")
                for dk in range(DT):
                    nc.tensor.matmul(
                        ps[:], x_chunk(s, dk, lt * 128, (lt + 1) * 128),
                        w[:, dk, :], start=(dk == 0), stop=(dk == DT - 1))
                t = _tt(pers, [128, HPG, HD + 1], BF16, f"v{s}_{lt}")
                nc.vector.tensor_copy(
                    t[:, :, 0:HD], ps[:].rearrange("p (h d) -> p h d", h=HPG))
                # mask column on gpsimd: these tiny copies would head-of-line
                # block the critical kz/qT casts on the in-order DVE queue.
                nc.gpsimd.tensor_copy(t[:, :, HD:HD + 1],
                                      m4_sb[s][:, lt, :, None])
                assert len(v_e[s]) == lt
                v_e[s].append(t)

            # ---- prefix: just enough projection for branch 0's first QK ----
            # q first: the first QK needs both qT casts but only the first
            # kz cast, so this ordering minimizes the DVE chain ahead of it.
            q_piece(1, 0, 0)
            q_piece(1, 0, 1, act=True)
            k_piece(1, 0, 0, act_h0=True)
            k_piece(1, 0, 1)

            # Remaining pieces, spliced into branches in pairs (pairs keep
            # the 2-slot "st" rotation parity so QK keeps its one-tile
            # lookahead ahead of the exp stream). Branch 0 takes all of v1
            # at kt 1/3/5/7; later branches take pairs at kt 2/4/6.
            splices = {
                0: [[lambda: v_piece(1, 0), lambda: v_piece(1, 1)],
                    [lambda: v_piece(1, 2), lambda: v_piece(1, 3)],
                    [lambda: v_piece(1, 4), lambda: v_piece(1, 5)],
                    [lambda: v_piece(1, 6), lambda: v_piece(1, 7)]],
                1: [[lambda: k_piece(1, 1, 0), lambda: k_piece(1, 1, 1)],
                    [lambda: q_piece(1, 1, 0), lambda: q_piece(1, 1, 1)]],
                2: [[lambda: q_piece(2, 0, 0), lambda: q_piece(2, 0, 1)],
                    [lambda: q_piece(2, 1, 0), lambda: q_piece(2, 1, 1)]],
                3: [[lambda: k_piece(2, 0, 0), lambda: k_piece(2, 0, 1)],
                    [lambda: k_piece(2, 1, 0), lambda: k_piece(2, 1, 1)]],
                4: [[lambda: v_piece(2, 0), lambda: v_piece(2, 1)],
                    [lambda: v_piece(2, 2), lambda: v_piece(2, 3)]],
                5: [[lambda: v_piece(2, 4), lambda: v_piece(2, 5)],
                    [lambda: v_piece(2, 6), lambda: v_piece(2, 7)]],
            }

            # ---- attention ----
            branches = [(ks, qs, h) for ks in (1, 2) for qs in (1, 2)
                        for h in range(HPG)]

            def emit_qk(ks, qs, h, kt, stage=False):
                st = _tt(stp, [128, L], F32, "st")
                blk = (h * KT + kt) * 128
                for nh in range(2):
                    nc.tensor.matmul(
                        st[:, nh * 512:(nh + 1) * 512],
                        kz[ks][:, blk:blk + 128],
                        qT[qs][:, h // 2, nh * 512:(nh + 1) * 512],
                        start=True,
                        stop=True,
                    )
                if stage:
                    # bounce the logits to SBUF: the exp reads SBUF at the
                    # same ACT cost, but the PSUM st slot frees as soon as
                    # the copy lands — deepening the QK lookahead beyond the
                    # two PSUM slots so projection-splice bursts on the PE
                    # don't starve the exp stream.
                    ss = smp.tile([128, L], F32, tag="sst", name="sst", bufs=3)
                    nc.vector.tensor_copy(ss[:], st[:])
                    return ss
                return st

            def emit_combine(p):
                # two branches deferred: by now the gpsimd broadcast of r is
                # long done, so these DVE ops never block the pipeline.
                ks, qs, h, acc, rbc = p
                oslice = outacc[qs][:, h, :]
                if ks == 1:
                    nc.vector.tensor_mul(oslice, acc[0:HD, :], rbc[:])
                else:
                    tmp = _tt(smp, [64, L], BF16, "tmp")
                    nc.vector.tensor_mul(tmp[:], acc[0:HD, :], rbc[:])
                    nc.vector.tensor_add(oslice, oslice, tmp[:])
                    nc.sync.dma_start(
                        out_d[qs][h * HD:(h + 1) * HD, :], oslice)

            def norm_pend(ks, qs, h, acc):
                # normalization scalar r = 1/denom in [1, L] (query mask and
                # the 0.5 averaging factor are applied on the host),
                # broadcast to 64 partitions on the (otherwise idle) gpsimd.
                s_sb = _tt(smp, [1, L], F32, "s_sb")
                nc.vector.tensor_copy(s_sb[:], acc[HD:HD + 1, :])
                rinv = _tt(smp, [1, L], F32, "rinv")
                nc.vector.reciprocal_approx_fast(rinv[:], s_sb[:])
                rbc = _tt(smp, [64, L], F32, "rbc")
                nc.gpsimd.partition_broadcast(rbc[:], rinv[:])
                return (ks, qs, h, acc, rbc)

            norm_q = []  # accs awaiting normalize (one branch old)
            combine_q = []  # pends awaiting combine (two branches old)
            look_st = None
            for bi, (ks, qs, h) in enumerate(branches):
                last = bi == len(branches) - 1
                spl = splices.get(bi, [])
                # QK for all 8 key tiles first: the PE free-runs one tile
                # ahead of ACT (throttled by the two st PSUM slots).
                sts = [look_st] if look_st is not None else []
                kts = (1, 3, 5, 7) if bi == 0 else (2, 4, 6)
                for kt in range(len(sts), KT):
                    sts.append(emit_qk(ks, qs, h, kt))
                    if kt in kts and spl:
                        for fn in spl.pop(0):
                            fn()
                pts = []
                for kt in range(KT):
                    if last and kt == KT - 1:
                        # split the final exp tile so the raw export of the
                        # first column half overlaps the second half.
                        for nhh in range(2):
                            pt = _tt(ptp, [128, 512], BF16, "pt")
                            nc.scalar.activation(
                                pt[:], sts[kt][:, nhh * 512:(nhh + 1) * 512],
                                EXP)
                            pts.append(pt)
                    else:
                        pt = _tt(ptp, [128, L], BF16, "pt")
                        nc.scalar.activation(pt[:], sts[kt][:], EXP)
                        pts.append(pt)
                # flush: combines of branch i-2, then normalizes of branch
                # i-1 — everything data-ready by now, so the in-order DVE
                # queue never parks on them.
                while combine_q:
                    emit_combine(combine_q.pop(0))
                while norm_q:
                    combine_q.append(norm_pend(*norm_q.pop(0)))
                if bi == 14:
                    # branches 14/15 skip the on-device combine: ship their
                    # ks=1 partners' halves now, raw accumulators later.
                    for hh in (2, 3):
                        nc.sync.dma_start(
                            out_d[2][hh * HD:(hh + 1) * HD, :],
                            outacc[2][:, hh, :])
                acc = _tt(accp, [HD + 1, L], F32, "acc")
                for kt in range(KT):
                    if kt == KT - 1 and not last:
                        # software-pipeline: next branch's first QK goes ahead
                        # of this branch's last PV so ACT rolls over gap-free.
                        look_st = emit_qk(*branches[bi + 1][:3], 0)
                    if last and kt == KT - 1:
                        for nh in range(2):
                            nc.tensor.matmul(
                                acc[:, nh * 512:(nh + 1) * 512],
                                v_e[ks][kt][:, h, :], pts[KT - 1 + nh][:],
                                start=False, stop=True)
                        for nh in range(2):
                            # raw export per column half: one DVE copy + DMA,
                            # no normalize chain in the kernel tail. The two
                            # halves ship on different queues.
                            sl = slice(nh * 512, (nh + 1) * 512)
                            o = _tt(smp, [HD + 1, L], F32, "oacc")
                            nc.vector.tensor_copy(o[:, sl], acc[:, sl])
                            # both halves on sync: a late SWDGE (gpsimd) DMA
                            # would extend the queue-drain in the epilogue.
                            nc.sync.dma_start(oa_d[15][:, sl], o[:, sl])
                    else:
                        for nh in range(2):
                            nc.tensor.matmul(
                                acc[:, nh * 512:(nh + 1) * 512],
                                v_e[ks][kt][:, h, :],
                                pts[kt][:, nh * 512:(nh + 1) * 512],
                                start=(kt == 0),
                                stop=(kt == KT - 1),
                            )
                if bi == 14:
                    o = _tt(smp, [HD + 1, L], F32, "oacc")
                    nc.vector.tensor_copy(o[:], acc[:])
                    nc.sync.dma_start(oa_d[14][:], o[:])
                elif not last:
                    norm_q.append((ks, qs, h, acc))

    nc.compile()
    return nc


def kernel(**inputs):
    global _NC
    if _NC is None:
        _NC = _build()

    mask1 = np.asarray(inputs["mask1"], dtype=np.float32)
    mask2 = np.asarray(inputs["mask2"], dtype=np.float32)
    # pre-zero masked tokens: masked keys then contribute exp(0)*0 = 0 to
    # both the attention numerator and (via the v mask column) denominator.
    x1 = np.asarray(inputs["input1"], dtype=np.float32) * mask1[:, :, None]
    x2 = np.asarray(inputs["input2"], dtype=np.float32) * mask2[:, :, None]
    W = {k: np.asarray(inputs[k], dtype=np.float32) for k in
         ("Wq1", "Wk1", "Wv1", "Wq2", "Wk2", "Wv2")}

    def pack_x(xb):
        # [L, D] -> xT [D, L] -> [p][dk][c] flattened
        return np.ascontiguousarray(
            xb.T.astype(np.float16).reshape(DT, 128, L)
            .transpose(1, 0, 2).reshape(128, DT * L))

    def pack_wqk(w, og):
        # W.T[:, og] is [D, OG] = [dk p][ot c128] -> [p][ot][dk][c]
        return np.ascontiguousarray(
            w.T[:, og].astype(np.float16).reshape(DT, 128, 2, 128)
            .transpose(1, 2, 0, 3).reshape(128, 2 * DT * 128))

    def pack_wv(w, og):
        # [dk p][c] -> [p][dk][c]
        return np.ascontiguousarray(
            w.T[:, og].astype(np.float16).reshape(DT, 128, OG)
            .transpose(1, 0, 2).reshape(128, DT * OG))

    in_maps = []
    for core in range(8):
        b, hg = core // 2, core % 2
        og = slice(hg * OG, (hg + 1) * OG)
        xp1 = pack_x(x1[b])
        wq1p = pack_wqk(W["Wq1"], og)
        wk1p = pack_wqk(W["Wk1"], og)
        m = {
            "x1a": np.ascontiguousarray(
                np.concatenate([xp1[:, :2 * L], wq1p[:, :DT * 128]], axis=1)),
            "x1b": np.ascontiguousarray(
                np.concatenate([xp1[:, 2 * L:], wk1p[:, :DT * 128]], axis=1)),
            "x2T": pack_x(x2[b]),
            "m41": np.ascontiguousarray(
                np.repeat(mask1[b].reshape(KT, 128).T[:, :, None], HPG, axis=2)
                .reshape(128, KT * HPG)),
            "m42": np.ascontiguousarray(
                np.repeat(mask2[b].reshape(KT, 128).T[:, :, None], HPG, axis=2)
                .reshape(128, KT * HPG)),
        }
        for s in (1, 2):
            m[f"wq{s}"] = pack_wqk(W[f"Wq{s}"], og)
            m[f"wk{s}"] = pack_wqk(W[f"Wk{s}"], og)
            m[f"wv{s}"] = pack_wv(W[f"Wv{s}"], og)
        in_maps.append(m)

    global LAST_RESULT
    if TRACE:
        _install_ntff_hook()
    res = run_bass_kernel_spmd(_NC, in_maps, list(range(8)), trace=TRACE)
    LAST_RESULT = res

    # query-side mask + the 0.5 branch-average factor are applied here (the
    # device ships out = sum_branches O/denom, unmasked).
    hm1 = (0.5 * mask1)[:, :, None]
    hm2 = (0.5 * mask2)[:, :, None]
    output1 = np.empty((NB, L, D), dtype=np.float32)
    output2 = np.empty((NB, L, D), dtype=np.float32)
    for core in range(8):
        b, hg = core // 2, core % 2
        og = slice(hg * OG, (hg + 1) * OG)
        output1[b, :, og] = np.asarray(res.results[core]["out1T"],
                                       dtype=np.float32).T
        o2 = np.asarray(res.results[core]["out2T"], dtype=np.float32)
        # branches 14/15 = (ks=2, qs=2, h=2/3) shipped raw accumulators;
        # divide and add their terms to the ks=1 halves in out2T.
        for bi, hh in ((14, 2), (15, 3)):
            a = np.asarray(res.results[core][f"oa{bi}"], dtype=np.float32)
            o2[hh * HD:(hh + 1) * HD, :] += a[:HD, :] / a[HD:HD + 1, :]
        output2[b, :, og] = o2.T
    output1 *= hm1
    output2 *= hm2
    return (output1, output2)


# revision 53
# speedup vs baseline: 1.0081x; 1.0052x over previous
# Trainium2 Bass kernel for the 4-branch cross-attention block.
#
# Problem: N=4 batches, L1=L2=1024, D=512, H=8 heads of 64.
#   q1,k1,v1 = proj(input1); q2,k2,v2 = proj(input2)
#   four attention branches (q1k1v1, q1k2v2, q2k1v1, q2k2v2), masked softmax
#   over the key axis, outputs averaged pairwise.
#
# Sharding: 8 cores = 4 batches x 2 head-groups (4 heads each). SPMD — one
# program, per-core data.
#
# Device-side dataflow (per core, 16 branch-heads of L x L attention):
#   ST   = K @ Q^T          (keys on partitions, queries on the free axis;
#                            lhsT = zero-padded kz block, rhs = qT, fp16)
#   P    = exp(ST)          (ACT engine; host pre-zeroed masked tokens in x,
#                            so masked keys give exp(0)=1 against v=0 rows
#                            and a masked ones-column — they drop out of both
#                            the numerator and the denominator exactly)
#   O^T  = [V | m]^T @ P    (bf16; mask column yields denominators in row 64)
#   r    = approx_recip(denom)   (DVE; query-mask and the 0.5 average factor
#                                 are applied on the host after gather)
#   rbc  = partition_broadcast(r)  (gpsimd)
#   out += O^T * rbc        (DVE, bf16 accumulator)
#
# Schedule (the exp stream on ACT, 128 tiles of [128,1024] @ ~1.03us = 132us,
# is the pipeline floor; PE total is within ~5% of it, so the projections
# must ride inside the stream's slack, not ahead of it):
#   - every input is host-packed so its DMA moves its full per-partition
#     span contiguously (DMA time is descriptor-rate bound, so fewer/bigger
#     descriptors win);
#   - launches go out in parallel on the SP/ACT/gpsimd queues, priority
#     ordered: the x1 halves ride two queues, wk1/wq1(ot0) right behind;
#   - the kz zero-fills run on gpsimd — anywhere else they head-of-line
#     block the projection casts (DVE and ACT queues are strictly in-order);
#   - the prefix projects k1/q1(ot0) + v1(lt0,lt1); the remaining 22 pieces
#     are spliced into branches 0-5 in PAIRS after kt 2/4/6 so the st-pool's
#     2-slot rotation parity (and with it the QK one-tile lookahead ahead of
#     ACT) survives;
#   - each branch's normalize chain is deferred one branch and its combine
#     two branches, so every DVE op is data-ready when the in-order DVE
#     queue reaches it — a waiting DVE op would stall the projection casts
#     and with them the QK slot rotation that feeds ACT;
#   - the last branch's kt7 exp + PV are split into column halves so its
#     normalize chain overlaps the final exps instead of trailing them.

import sys

sys.path.insert(0, "/opt/trn_rl_repo")

import ml_dtypes
import numpy as np

import concourse.bacc as bacc
import concourse.mybir as mybir
import concourse.tile as tile
from concourse.bass_utils import run_bass_kernel_spmd

F32 = mybir.dt.float32
F16 = mybir.dt.float16
BF16 = mybir.dt.bfloat16
EXP = mybir.ActivationFunctionType.Exp

L = 1024  # sequence length (both sides)
D = 512  # hidden
NB = 4  # batches
HPG = 4  # heads per core (head group)
HD = 64  # head size
OG = HPG * HD  # output channels per core = 256
KT = L // 128  # 8 key tiles
DT = D // 128  # 4 contraction tiles for projections

_NC = None  # cached compiled program
TRACE = False  # set by test harness to capture an NTFF profile
LAST_RESULT = None  # full BassKernelResults of the last run (for profiling)


def _tt(pool, shape, dtype, tag):
    return pool.tile(shape, dtype, tag=tag, name=tag)


def _install_ntff_hook():
    # antenv.axon_hooks is absent in this image; provide it so
    # run_bass_kernel_spmd(trace=True) can capture NTFF profiles.
    import types, contextlib, ctypes

    if "antenv.axon_hooks" in sys.modules:
        return
    lib = ctypes.CDLL("/opt/axon/libaxon_pjrt.so")
    lib.axon_start_nrt_profile.argtypes = [
        ctypes.POINTER(ctypes.c_int64),
        ctypes.c_size_t,
    ]
    lib.axon_start_nrt_profile.restype = ctypes.c_int64
    lib.axon_stop_nrt_profile.argtypes = [ctypes.c_char_p]
    lib.axon_stop_nrt_profile.restype = ctypes.c_int64

    @contextlib.contextmanager
    def _hook(output_dir, device_ids):
        import jax

        jax.devices()
        if device_ids:
            ids = (ctypes.c_int64 * len(device_ids))(*device_ids)
            rc = lib.axon_start_nrt_profile(ids, len(device_ids))
        else:
            rc = lib.axon_start_nrt_profile(None, 0)
        if rc != 0:
            raise RuntimeError(f"axon_start_nrt_profile rc={rc}")
        try:
            yield
        finally:
            n = lib.axon_stop_nrt_profile(str(output_dir).encode())
            print(f"ntff profile: {n} file(s) in {output_dir}", file=sys.stderr)

    mod = types.ModuleType("antenv.axon_hooks")
    mod.get_axon_ntff_profile_hook = lambda: _hook
    mod.set_axon_ntff_profile_hook = lambda h: None
    sys.modules["antenv.axon_hooks"] = mod


def _build():
    nc = bacc.Bacc("TRN2", target_bir_lowering=False, debug=False, num_devices=8)

    # x and weights arrive as fp16 (host-converted): halves input DMA and
    # makes every projection LDWEIGHTS a cheap 2-byte load, at ~8x finer
    # quantization than bf16 (which overshoots the 2e-2 error budget).
    # side 1 arrives as two packed tensors [x-half | critical ot0 weights]
    # so the whole branch-0 prefix is gated by exactly two parallel DMAs.
    x1a_d = nc.declare_dram_parameter("x1a", [128, 2 * L + DT * 128], F16,
                                      isOutput=False)
    x1b_d = nc.declare_dram_parameter("x1b", [128, 2 * L + DT * 128], F16,
                                      isOutput=False)
    x_d = {2: nc.declare_dram_parameter("x2T", [128, DT * L], F16,
                                        isOutput=False)}
    ws = {}
    for wn in ("wq1", "wk1", "wq2", "wk2"):
        # [p][ot][dk][c128]
        ws[wn] = nc.declare_dram_parameter(wn, [128, 2 * DT * 128], F16,
                                           isOutput=False)
    for wn in ("wv1", "wv2"):
        # [p][dk][c256]
        ws[wn] = nc.declare_dram_parameter(wn, [128, DT * OG], F16,
                                           isOutput=False)
    m4_d = {s: nc.declare_dram_parameter(f"m4{s}", [128, KT * HPG], F32,
                                         isOutput=False) for s in (1, 2)}
    out_d = {s: nc.declare_dram_parameter(f"out{s}T", [OG, L], BF16, isOutput=True)
             for s in (1, 2)}
    # branches 14/15 ship raw f32 accumulators (numerator + denominator
    # row) instead of normalizing on-device — their normalize chains would
    # otherwise trail the exp stream serially. The host divides.
    oa_d = {bi: nc.declare_dram_parameter(f"oa{bi}", [HD + 1, L], F32,
                                          isOutput=True) for bi in (14, 15)}

    with tile.TileContext(nc) as tc:
        with (
            tc.tile_pool(name="pers", bufs=1) as pers,
            tc.tile_pool(name="pt", bufs=20) as ptp,
            tc.tile_pool(name="sm", bufs=2) as smp,
            tc.tile_pool(name="st", bufs=2, space="PSUM") as stp,
            tc.tile_pool(name="acc", bufs=2, space="PSUM") as accp,
        ):
            # ---- persistent tiles ----
            x_r, w_r, m4_sb, kz, qT = {}, {}, {}, {}, {}
            v_e = {1: [], 2: []}
            x1ab = {0: _tt(pers, [128, 2 * L + DT * 128], F16, "x1a"),
                    1: _tt(pers, [128, 2 * L + DT * 128], F16, "x1b")}
            for s in (1, 2):
                if s == 2:
                    x_r[s] = _tt(pers, [128, DT, L], F16, f"x{s}")
                m4_sb[s] = _tt(pers, [128, KT, HPG], F32, f"m4{s}")
                kz[s] = _tt(pers, [128, HPG * KT * 128], F16, f"kz{s}")
                qT[s] = _tt(pers, [128, 2, L], F16, f"q{s}T")
            for wn in ("wq1", "wk1", "wq2", "wk2"):
                w_r[wn] = _tt(pers, [128, 2, DT, 128], F16, wn)
            for wn in ("wv1", "wv2"):
                w_r[wn] = _tt(pers, [128, DT, OG], F16, wn)
            outacc = {qs: _tt(pers, [HD, HPG, L], BF16, f"out{qs}")
                      for qs in (1, 2)}

            junk = _tt(pers, [128, 512], F16, "junk")
            nc.vector.memset(junk[:], 1.0)
            junk_e = _tt(pers, [128, 512], BF16, "junk_e")

            # ---- input DMA: parallel queues, priority order ----
            # HWDGE queues are SP + ACT only; gpsimd launches via SWDGE and
            # also hosts the kz zero-fills (it is otherwise idle early).
            def _w_dma(eng, wn, ot):
                eng.dma_start(
                    w_r[wn][:, ot, :, :],
                    ws[wn][:, ot * 512:(ot + 1) * 512]
                    .rearrange("p (dk c) -> p dk c", c=128))

            def _w_dma_all(eng, wn):
                eng.dma_start(
                    w_r[wn][:],
                    ws[wn][:].rearrange("p (ot dk c) -> p ot dk c", ot=2,
                                        c=128))

            def _x_dma(eng, s, half):
                assert s == 2
                eng.dma_start(
                    x_r[s][:, 2 * half:2 * half + 2, :],
                    x_d[s][:, 2048 * half:2048 * half + 2048]
                    .rearrange("p (dk c) -> p dk c", c=L))

            def _wv_dma(eng, s):
                eng.dma_start(w_r[f"wv{s}"][:],
                              ws[f"wv{s}"][:].rearrange("p (dk c) -> p dk c",
                                                        c=OG))

            # Only the branch-0 critical set launches up front — everything
            # else is deferred into the branch loop so the DMA engines spend
            # the head exclusively on x1/wk1/wq1/wv1. (Each queue serializes
            # its transfers at ~1.7us fixed + bytes at ~140GB/s, and the
            # engine pool is shared across queues.)
            nc.sync.dma_start(x1ab[0][:], x1a_d[:])
            nc.scalar.dma_start(x1ab[1][:], x1b_d[:])
            _wv_dma(nc.gpsimd, 1)
            nc.gpsimd.dma_start(m4_sb[1][:].rearrange("p a b -> p (a b)"),
                                m4_d[1][:])
            # exp-table preload + ACT warm-up while the inputs stream in
            nc.scalar.activation(junk_e[:], junk[:], EXP)
            nc.gpsimd.memset(kz[1][:], 0.0)

            # The scheduler front-loads dependency-free DMA launches, so the
            # non-critical transfers would otherwise compete with x1 for the
            # shared DMA engines. Gate each one on x1's arrival by writing a
            # corner of its destination (WAW dep) sourced from x1b.
            gq = x1ab[1][0:1, 0:1]
            for corner in (
                x_r[2][0:1, 0, 0:1], x_r[2][0:1, 2, 0:1],
                w_r["wk1"][0:1, 1, 0, 0:1], w_r["wq1"][0:1, 1, 0, 0:1],
                w_r["wv2"][0:1, 0, 0:1],
                w_r["wq2"][0:1, 0, 0, 0:1], w_r["wk2"][0:1, 0, 0, 0:1],
            ):
                nc.gpsimd.tensor_copy(corner, gq)
            _w_dma(nc.gpsimd, "wk1", 1)
            _w_dma(nc.gpsimd, "wq1", 1)
            _x_dma(nc.sync, 2, 0)
            _x_dma(nc.sync, 2, 1)
            _wv_dma(nc.sync, 2)
            _w_dma_all(nc.sync, "wq2")
            _w_dma_all(nc.sync, "wk2")
            nc.gpsimd.dma_start(m4_sb[2][:].rearrange("p a b -> p (a b)"),
                                m4_d[2][:])
            nc.gpsimd.memset(kz[2][:], 0.0)

            # PE warm-up: the HAM clock gate releases only after ~3.4us of
            # sustained activity; these bridge the DMA window so the first
            # projections run at full rate.
            for _ in range(8):
                wps = _tt(stp, [128, 512], F32, "st")
                nc.tensor.matmul(wps[:], junk[:, 0:128], junk[:], start=True,
                                 stop=True)

            # ---- projection pieces ([128, 512] PSUM chunks) ----
            # qT per side: [128, 2, L] (tile ht holds heads 2ht, 2ht+1).
            # kz per side: [128, HPG*KT*128] zero-padded per (head, kt) block
            # so QK's moving qT streams all 128 partitions at full rate.
            def x_chunk(s, dk, c0, c1):
                # side 1's x lives in the packed x1a/x1b tiles
                if s == 1:
                    t, loc = x1ab[dk // 2], dk % 2
                    return t[:, loc * L + c0:loc * L + c1]
                return x_r[s][:, dk, c0:c1]

            def w_chunk(kind, s, ot, dk):
                if s == 1 and ot == 0:
                    t = x1ab[0] if kind == "q" else x1ab[1]
                    return t[:, 2 * L + dk * 128:2 * L + (dk + 1) * 128]
                return w_r[f"w{kind}{s}"][:, ot, dk, :]

            def k_piece(s, ot, nh, act_h0=False):
                # output chans = heads 2ot,2ot+1 ; keys nh*512:(nh+1)*512
                ps = _tt(stp, [128, 512], F32, "st")
                for dk in range(DT):
                    nc.tensor.matmul(
                        ps[:], w_chunk("k", s, ot, dk),
                        x_chunk(s, dk, nh * 512, (nh + 1) * 512),
                        start=(dk == 0), stop=(dk == DT - 1))
                for hh in range(2):
                    h = 2 * ot + hh
                    po = hh * 64
                    base = h * KT * 128 + nh * 512
                    # the pre-stream head-0 cast can ride the idle ACT engine
                    # to shorten the DVE chain ahead of the first QK
                    eng = nc.scalar.copy if (act_h0 and hh == 0) else \
                        nc.vector.tensor_copy
                    eng(kz[s][po:po + 64, base:base + 512], ps[po:po + 64, :])

            def q_piece(s, ot, nh, act=False):
                ps = _tt(stp, [128, 512], F32, "st")
                for dk in range(DT):
                    nc.tensor.matmul(
                        ps[:], w_chunk("q", s, ot, dk),
                        x_chunk(s, dk, nh * 512, (nh + 1) * 512),
                        start=(dk == 0), stop=(dk == DT - 1))
                eng = nc.scalar.copy if act else nc.vector.tensor_copy
                eng(qT[s][:, ot, nh * 512:(nh + 1) * 512], ps[:])

            # v in natural layout with mask column: [128, HPG, 65] per key tile
            def v_piece(s, lt):
                w = w_r[f"wv{s}"]
                ps = _tt(stp, [128, OG], F32, "st")
                for dk in range(DT):
                    nc.tensor.matmul(
                        ps[:], x_chunk(s, dk, lt * 128, (lt + 1) * 128),
                        w[:, dk, :], start=(dk == 0), stop=(dk == DT - 1))
                t = _tt(pers, [128, HPG, HD + 1], BF16, f"v{s}_{lt}")
                nc.vector.tensor_copy(
                    t[:, :, 0:HD], ps[:].rearrange("p (h d) -> p h d", h=HPG))
                # mask column on gpsimd: these tiny copies would head-of-line
                # block the critical kz/qT casts on the in-order DVE queue.
                nc.gpsimd.tensor_copy(t[:, :, HD:HD + 1],
                                      m4_sb[s][:, lt, :, None])
                assert len(v_e[s]) == lt
                v_e[s].append(t)

            # ---- prefix: just enough projection for branch 0's first QK ----
            # q first: the first QK needs both qT casts but only the first
            # kz cast, so this ordering minimizes the DVE chain ahead of it.
            q_piece(1, 0, 0)
            q_piece(1, 0, 1, act=True)
            k_piece(1, 0, 0, act_h0=True)
            k_piece(1, 0, 1)

            # Remaining pieces, spliced into branches in pairs (pairs keep
            # the 2-slot "st" rotation parity so QK keeps its one-tile
            # lookahead ahead of the exp stream). Branch 0 takes all of v1
            # at kt 1/3/5/7; later branches take pairs at kt 2/4/6.
            splices = {
                0: [[lambda: v_piece(1, 0), lambda: v_piece(1, 1)],
                    [lambda: v_piece(1, 2), lambda: v_piece(1, 3)],
                    [lambda: v_piece(1, 4), lambda: v_piece(1, 5)],
                    [lambda: v_piece(1, 6), lambda: v_piece(1, 7)]],
                1: [[lambda: k_piece(1, 1, 0), lambda: k_piece(1, 1, 1)],
                    [lambda: q_piece(1, 1, 0), lambda: q_piece(1, 1, 1)]],
                2: [[lambda: q_piece(2, 0, 0), lambda: q_piece(2, 0, 1)],
                    [lambda: q_piece(2, 1, 0), lambda: q_piece(2, 1, 1)]],
                3: [[lambda: k_piece(2, 0, 0), lambda: k_piece(2, 0, 1)],
                    [lambda: k_piece(2, 1, 0), lambda: k_piece(2, 1, 1)]],
                4: [[lambda: v_piece(2, 0), lambda: v_piece(2, 1)],
                    [lambda: v_piece(2, 2), lambda: v_piece(2, 3)]],
                5: [[lambda: v_piece(2, 4), lambda: v_piece(2, 5)],
                    [lambda: v_piece(2, 6), lambda: v_piece(2, 7)]],
            }

            # ---- attention ----
            branches = [(ks, qs, h) for ks in (1, 2) for qs in (1, 2)
                        for h in range(HPG)]

            def emit_qk(ks, qs, h, kt, stage=False):
                st = _tt(stp, [128, L], F32, "st")
                blk = (h * KT + kt) * 128
                for nh in range(2):
                    nc.tensor.matmul(
                        st[:, nh * 512:(nh + 1) * 512],
                        kz[ks][:, blk:blk + 128],
                        qT[qs][:, h // 2, nh * 512:(nh + 1) * 512],
                        start=True,
                        stop=True,
                    )
                if stage:
                    # bounce the logits to SBUF: the exp reads SBUF at the
                    # same ACT cost, but the PSUM st slot frees as soon as
                    # the copy lands — deepening the QK lookahead beyond the
                    # two PSUM slots so projection-splice bursts on the PE
                    # don't starve the exp stream.
                    ss = smp.tile([128, L], F32, tag="sst", name="sst", bufs=3)
                    nc.vector.tensor_copy(ss[:], st[:])
                    return ss
                return st

            def emit_combine(p):
                # two branches deferred: by now the gpsimd broadcast of r is
                # long done, so these DVE ops never block the pipeline.
                ks, qs, h, acc, rbc = p
                oslice = outacc[qs][:, h, :]
                if ks == 1:
                    nc.vector.tensor_mul(oslice, acc[0:HD, :], rbc[:])
                else:
                    tmp = _tt(smp, [64, L], BF16, "tmp")
                    nc.vector.tensor_mul(tmp[:], acc[0:HD, :], rbc[:])
                    nc.vector.tensor_add(oslice, oslice, tmp[:])
                    nc.sync.dma_start(
                        out_d[qs][h * HD:(h + 1) * HD, :], oslice)

            def norm_pend(ks, qs, h, acc):
                # normalization scalar r = 1/denom in [1, L] (query mask and
                # the 0.5 averaging factor are applied on the host),
                # broadcast to 64 partitions on the (otherwise idle) gpsimd.
                s_sb = _tt(smp, [1, L], F32, "s_sb")
                nc.vector.tensor_copy(s_sb[:], acc[HD:HD + 1, :])
                rinv = _tt(smp, [1, L], F32, "rinv")
                nc.vector.reciprocal_approx_fast(rinv[:], s_sb[:])
                rbc = _tt(smp, [64, L], F32, "rbc")
                nc.gpsimd.partition_broadcast(rbc[:], rinv[:])
                return (ks, qs, h, acc, rbc)

            norm_q = []  # accs awaiting normalize (one branch old)
            combine_q = []  # pends awaiting combine (two branches old)
            look_st = None
            for bi, (ks, qs, h) in enumerate(branches):
                last = bi == len(branches) - 1
                spl = splices.get(bi, [])
                # QK for all 8 key tiles first: the PE free-runs one tile
                # ahead of ACT (throttled by the two st PSUM slots).
                sts = [look_st] if look_st is not None else []
                kts = (1, 3, 5, 7) if bi == 0 else (2, 4, 6)
                for kt in range(len(sts), KT):
                    sts.append(emit_qk(ks, qs, h, kt))
                    if kt in kts and spl:
                        for fn in spl.pop(0):
                            fn()
                pts = []
                for kt in range(KT):
                    if last and kt == KT - 1:
                        # split the final exp tile so the raw export of the
                        # first column half overlaps the second half.
                        for nhh in range(2):
                            pt = _tt(ptp, [128, 512], BF16, "pt")
                            nc.scalar.activation(
                                pt[:], sts[kt][:, nhh * 512:(nhh + 1) * 512],
                                EXP)
                            pts.append(pt)
                    else:
                        pt = _tt(ptp, [128, L], BF16, "pt")
                        nc.scalar.activation(pt[:], sts[kt][:], EXP)
                        pts.append(pt)
                # flush: combines of branch i-2, then normalizes of branch
                # i-1 — everything data-ready by now, so the in-order DVE
                # queue never parks on them.
                while combine_q:
                    emit_combine(combine_q.pop(0))
                while norm_q:
                    combine_q.append(norm_pend(*norm_q.pop(0)))
                if bi == 14:
                    # branches 14/15 skip the on-device combine: ship their
                    # ks=1 partners' halves now, raw accumulators later.
                    for hh in (2, 3):
                        nc.sync.dma_start(
                            out_d[2][hh * HD:(hh + 1) * HD, :],
                            outacc[2][:, hh, :])
                acc = _tt(accp, [HD + 1, L], F32, "acc")
                for kt in range(KT):
                    if kt == KT - 1 and not last:
                        # software-pipeline: next branch's first QK goes ahead
                        # of this branch's last PV so ACT rolls over gap-free.
                        look_st = emit_qk(*branches[bi + 1][:3], 0)
                    if last and kt == KT - 1:
                        for nh in range(2):
                            nc.tensor.matmul(
                                acc[:, nh * 512:(nh + 1) * 512],
                                v_e[ks][kt][:, h, :], pts[KT - 1 + nh][:],
                                start=False, stop=True)
                        for nh in range(2):
                            # raw export per column half: one DVE copy + DMA,
                            # no normalize chain in the kernel tail. The two
                            # halves ship on different queues.
                            sl = slice(nh * 512, (nh + 1) * 512)
                            o = _tt(smp, [HD + 1, L], F32, "oacc")
                            nc.vector.tensor_copy(o[:, sl], acc[:, sl])
                            eng = nc.sync if nh == 0 else nc.gpsimd
                            eng.dma_start(oa_d[15][:, sl], o[:, sl])
                    else:
                        for nh in range(2):
                            nc.tensor.matmul(
                                acc[:, nh * 512:(nh + 1) * 512],
                                v_e[ks][kt][:, h, :],
                                pts[kt][:, nh * 512:(nh + 1) * 512],
                                start=(kt == 0),
                                stop=(kt == KT - 1),
                            )
                if bi == 14:
                    o = _tt(smp, [HD + 1, L], F32, "oacc")
                    nc.vector.tensor_copy(o[:], acc[:])
                    nc.sync.dma_start(oa_d[14][:], o[:])
                elif not last:
                    norm_q.append((ks, qs, h, acc))

    nc.compile()
    return nc


def kernel(**inputs):
    global _NC
    if _NC is None:
        _NC = _build()

    mask1 = np.asarray(inputs["mask1"], dtype=np.float32)
    mask2 = np.asarray(inputs["mask2"], dtype=np.float32)
    # pre-zero masked tokens: masked keys then contribute exp(0)*0 = 0 to
    # both the attention numerator and (via the v mask column) denominator.
    x1 = np.asarray(inputs["input1"], dtype=np.float32) * mask1[:, :, None]
    x2 = np.asarray(inputs["input2"], dtype=np.float32) * mask2[:, :, None]
    W = {k: np.asarray(inputs[k], dtype=np.float32) for k in
         ("Wq1", "Wk1", "Wv1", "Wq2", "Wk2", "Wv2")}

    def pack_x(xb):
        # [L, D] -> xT [D, L] -> [p][dk][c] flattened
        return np.ascontiguousarray(
            xb.T.astype(np.float16).reshape(DT, 128, L)
            .transpose(1, 0, 2).reshape(128, DT * L))

    def pack_wqk(w, og):
        # W.T[:, og] is [D, OG] = [dk p][ot c128] -> [p][ot][dk][c]
        return np.ascontiguousarray(
            w.T[:, og].astype(np.float16).reshape(DT, 128, 2, 128)
            .transpose(1, 2, 0, 3).reshape(128, 2 * DT * 128))

    def pack_wv(w, og):
        # [dk p][c] -> [p][dk][c]
        return np.ascontiguousarray(
            w.T[:, og].astype(np.float16).reshape(DT, 128, OG)
            .transpose(1, 0, 2).reshape(128, DT * OG))

    in_maps = []
    for core in range(8):
        b, hg = core // 2, core % 2
        og = slice(hg * OG, (hg + 1) * OG)
        xp1 = pack_x(x1[b])
        wq1p = pack_wqk(W["Wq1"], og)
        wk1p = pack_wqk(W["Wk1"], og)
        m = {
            "x1a": np.ascontiguousarray(
                np.concatenate([xp1[:, :2 * L], wq1p[:, :DT * 128]], axis=1)),
            "x1b": np.ascontiguousarray(
                np.concatenate([xp1[:, 2 * L:], wk1p[:, :DT * 128]], axis=1)),
            "x2T": pack_x(x2[b]),
            "m41": np.ascontiguousarray(
                np.repeat(mask1[b].reshape(KT, 128).T[:, :, None], HPG, axis=2)
                .reshape(128, KT * HPG)),
            "m42": np.ascontiguousarray(
                np.repeat(mask2[b].reshape(KT, 128).T[:, :, None], HPG, axis=2)
                .reshape(128, KT * HPG)),
        }
        for s in (1, 2):
            m[f"wq{s}"] = pack_wqk(W[f"Wq{s}"], og)
            m[f"wk{s}"] = pack_wqk(W[f"Wk{s}"], og)
            m[f"wv{s}"] = pack_wv(W[f"Wv{s}"], og)
        in_maps.append(m)

    global LAST_RESULT
    if TRACE:
        _install_ntff_hook()
    res = run_bass_kernel_spmd(_NC, in_maps, list(range(8)), trace=TRACE)
    LAST_RESULT = res

    # query-side mask + the 0.5 branch-average factor are applied here (the
    # device ships out = sum_branches O/denom, unmasked).
    hm1 = (0.5 * mask1)[:, :, None]
    hm2 = (0.5 * mask2)[:, :, None]
    output1 = np.empty((NB, L, D), dtype=np.float32)
    output2 = np.empty((NB, L, D), dtype=np.float32)
    for core in range(8):
        b, hg = core // 2, core % 2
        og = slice(hg * OG, (hg + 1) * OG)
        output1[b, :, og] = np.asarray(res.results[core]["out1T"],
                                       dtype=np.float32).T
        o2 = np.asarray(res.results[core]["out2T"], dtype=np.float32)
        # branches 14/15 = (ks=2, qs=2, h=2/3) shipped raw accumulators;
        # divide and add their terms to the ks=1 halves in out2T.
        for bi, hh in ((14, 2), (15, 3)):
            a = np.asarray(res.results[core][f"oa{bi}"], dtype=np.float32)
            o2[hh * HD:(hh + 1) * HD, :] += a[:HD, :] / a[HD:HD + 1, :]
        output2[b, :, og] = o2.T
    output1 *= hm1
    output2 *= hm2
    return (output1, output2)


# revision 56
# speedup vs baseline: 1.0187x; 1.0105x over previous
# Trainium2 Bass kernel for the 4-branch cross-attention block.
#
# Problem: N=4 batches, L1=L2=1024, D=512, H=8 heads of 64.
#   q1,k1,v1 = proj(input1); q2,k2,v2 = proj(input2)
#   four attention branches (q1k1v1, q1k2v2, q2k1v1, q2k2v2), masked softmax
#   over the key axis, outputs averaged pairwise.
#
# Sharding: 8 cores = 4 batches x 2 head-groups (4 heads each). SPMD — one
# program, per-core data.
#
# Device-side dataflow (per core, 16 branch-heads of L x L attention):
#   ST   = K @ Q^T          (keys on partitions, queries on the free axis;
#                            lhsT = zero-padded kz block, rhs = qT, fp16)
#   P    = exp(ST)          (ACT engine; host pre-zeroed masked tokens in x,
#                            so masked keys give exp(0)=1 against v=0 rows
#                            and a masked ones-column — they drop out of both
#                            the numerator and the denominator exactly)
#   O^T  = [V | m]^T @ P    (bf16; mask column yields denominators in row 64)
#   r    = approx_recip(denom)   (DVE; query-mask and the 0.5 average factor
#                                 are applied on the host after gather)
#   rbc  = partition_broadcast(r)  (gpsimd)
#   out += O^T * rbc        (DVE, bf16 accumulator)
#
# Schedule (the exp stream on ACT, 128 tiles of [128,1024] @ ~1.03us = 132us,
# is the pipeline floor; PE total is within ~5% of it, so the projections
# must ride inside the stream's slack, not ahead of it):
#   - every input is host-packed so its DMA moves its full per-partition
#     span contiguously (DMA time is descriptor-rate bound, so fewer/bigger
#     descriptors win);
#   - launches go out in parallel on the SP/ACT/gpsimd queues, priority
#     ordered: the x1 halves ride two queues, wk1/wq1(ot0) right behind;
#   - the kz zero-fills run on gpsimd — anywhere else they head-of-line
#     block the projection casts (DVE and ACT queues are strictly in-order);
#   - the prefix projects k1/q1(ot0) + v1(lt0,lt1); the remaining 22 pieces
#     are spliced into branches 0-5 in PAIRS after kt 2/4/6 so the st-pool's
#     2-slot rotation parity (and with it the QK one-tile lookahead ahead of
#     ACT) survives;
#   - each branch's normalize chain is deferred one branch and its combine
#     two branches, so every DVE op is data-ready when the in-order DVE
#     queue reaches it — a waiting DVE op would stall the projection casts
#     and with them the QK slot rotation that feeds ACT;
#   - the last branch's kt7 exp + PV are split into column halves so its
#     normalize chain overlaps the final exps instead of trailing them.

import sys

sys.path.insert(0, "/opt/trn_rl_repo")

import ml_dtypes
import numpy as np

import concourse.bacc as bacc
import concourse.mybir as mybir
import concourse.tile as tile
from concourse.bass_utils import run_bass_kernel_spmd

F32 = mybir.dt.float32
F16 = mybir.dt.float16
BF16 = mybir.dt.bfloat16
EXP = mybir.ActivationFunctionType.Exp

L = 1024  # sequence length (both sides)
D = 512  # hidden
NB = 4  # batches
HPG = 4  # heads per core (head group)
HD = 64  # head size
OG = HPG * HD  # output channels per core = 256
KT = L // 128  # 8 key tiles
DT = D // 128  # 4 contraction tiles for projections

_NC = None  # cached compiled program
TRACE = False  # set by test harness to capture an NTFF profile
LAST_RESULT = None  # full BassKernelResults of the last run (for profiling)


def _tt(pool, shape, dtype, tag):
    return pool.tile(shape, dtype, tag=tag, name=tag)


def _install_ntff_hook():
    # antenv.axon_hooks is absent in this image; provide it so
    # run_bass_kernel_spmd(trace=True) can capture NTFF profiles.
    import types, contextlib, ctypes

    if "antenv.axon_hooks" in sys.modules:
        return
    lib = ctypes.CDLL("/opt/axon/libaxon_pjrt.so")
    lib.axon_start_nrt_profile.argtypes = [
        ctypes.POINTER(ctypes.c_int64),
        ctypes.c_size_t,
    ]
    lib.axon_start_nrt_profile.restype = ctypes.c_int64
    lib.axon_stop_nrt_profile.argtypes = [ctypes.c_char_p]
    lib.axon_stop_nrt_profile.restype = ctypes.c_int64

    @contextlib.contextmanager
    def _hook(output_dir, device_ids):
        import jax

        jax.devices()
        if device_ids:
            ids = (ctypes.c_int64 * len(device_ids))(*device_ids)
            rc = lib.axon_start_nrt_profile(ids, len(device_ids))
        else:
            rc = lib.axon_start_nrt_profile(None, 0)
        if rc != 0:
            raise RuntimeError(f"axon_start_nrt_profile rc={rc}")
        try:
            yield
        finally:
            n = lib.axon_stop_nrt_profile(str(output_dir).encode())
            print(f"ntff profile: {n} file(s) in {output_dir}", file=sys.stderr)

    mod = types.ModuleType("antenv.axon_hooks")
    mod.get_axon_ntff_profile_hook = lambda: _hook
    mod.set_axon_ntff_profile_hook = lambda h: None
    sys.modules["antenv.axon_hooks"] = mod


def _build():
    nc = bacc.Bacc("TRN2", target_bir_lowering=False, debug=False, num_devices=8)

    # x and weights arrive as fp16 (host-converted): halves input DMA and
    # makes every projection LDWEIGHTS a cheap 2-byte load, at ~8x finer
    # quantization than bf16 (which overshoots the 2e-2 error budget).
    # side 1 arrives as two packed tensors [x-half | critical ot0 weights]
    # so the whole branch-0 prefix is gated by exactly two parallel DMAs.
    x1a_d = nc.declare_dram_parameter("x1a", [128, 2 * L + DT * 128], F16,
                                      isOutput=False)
    x1b_d = nc.declare_dram_parameter("x1b", [128, 2 * L + DT * 128], F16,
                                      isOutput=False)
    x_d = {2: nc.declare_dram_parameter("x2T", [128, DT * L], F16,
                                        isOutput=False)}
    ws = {}
    for wn in ("wq1", "wk1", "wq2", "wk2"):
        # [p][ot][dk][c128]
        ws[wn] = nc.declare_dram_parameter(wn, [128, 2 * DT * 128], F16,
                                           isOutput=False)
    for wn in ("wv1", "wv2"):
        # [p][dk][c256]
        ws[wn] = nc.declare_dram_parameter(wn, [128, DT * OG], F16,
                                           isOutput=False)
    m4_d = {s: nc.declare_dram_parameter(f"m4{s}", [128, KT * HPG], F32,
                                         isOutput=False) for s in (1, 2)}
    out_d = {s: nc.declare_dram_parameter(f"out{s}T", [OG, L], BF16, isOutput=True)
             for s in (1, 2)}
    # branches 14/15 ship raw f32 accumulators (numerator + denominator
    # row) instead of normalizing on-device — their normalize chains would
    # otherwise trail the exp stream serially. The host divides.
    oa_d = {bi: nc.declare_dram_parameter(f"oa{bi}", [HD + 1, L], F32,
                                          isOutput=True) for bi in (14, 15)}

    with tile.TileContext(nc) as tc:
        with (
            tc.tile_pool(name="pers", bufs=1) as pers,
            tc.tile_pool(name="pt", bufs=20) as ptp,
            tc.tile_pool(name="sm", bufs=2) as smp,
            tc.tile_pool(name="st", bufs=2, space="PSUM") as stp,
            tc.tile_pool(name="acc", bufs=2, space="PSUM") as accp,
        ):
            # ---- persistent tiles ----
            x_r, w_r, m4_sb, kz, qT = {}, {}, {}, {}, {}
            v_e = {1: [], 2: []}
            x1ab = {0: _tt(pers, [128, 2 * L + DT * 128], F16, "x1a"),
                    1: _tt(pers, [128, 2 * L + DT * 128], F16, "x1b")}
            for s in (1, 2):
                if s == 2:
                    x_r[s] = _tt(pers, [128, DT, L], F16, f"x{s}")
                m4_sb[s] = _tt(pers, [128, KT, HPG], F32, f"m4{s}")
                kz[s] = _tt(pers, [128, HPG * KT * 128], F16, f"kz{s}")
                qT[s] = _tt(pers, [128, 2, L], F16, f"q{s}T")
            for wn in ("wq1", "wk1", "wq2", "wk2"):
                w_r[wn] = _tt(pers, [128, 2, DT, 128], F16, wn)
            for wn in ("wv1", "wv2"):
                w_r[wn] = _tt(pers, [128, DT, OG], F16, wn)
            outacc = {qs: _tt(pers, [HD, HPG, L], BF16, f"out{qs}")
                      for qs in (1, 2)}

            junk = _tt(pers, [128, 512], F16, "junk")
            nc.vector.memset(junk[:], 1.0)
            junk_e = _tt(pers, [128, 512], BF16, "junk_e")

            # ---- input DMA: parallel queues, priority order ----
            # HWDGE queues are SP + ACT only; gpsimd launches via SWDGE and
            # also hosts the kz zero-fills (it is otherwise idle early).
            def _w_dma(eng, wn, ot):
                eng.dma_start(
                    w_r[wn][:, ot, :, :],
                    ws[wn][:, ot * 512:(ot + 1) * 512]
                    .rearrange("p (dk c) -> p dk c", c=128))

            def _w_dma_all(eng, wn):
                eng.dma_start(
                    w_r[wn][:],
                    ws[wn][:].rearrange("p (ot dk c) -> p ot dk c", ot=2,
                                        c=128))

            def _x_dma(eng, s, half):
                assert s == 2
                eng.dma_start(
                    x_r[s][:, 2 * half:2 * half + 2, :],
                    x_d[s][:, 2048 * half:2048 * half + 2048]
                    .rearrange("p (dk c) -> p dk c", c=L))

            def _wv_dma(eng, s):
                eng.dma_start(w_r[f"wv{s}"][:],
                              ws[f"wv{s}"][:].rearrange("p (dk c) -> p dk c",
                                                        c=OG))

            # Only the branch-0 critical set launches up front — everything
            # else is deferred into the branch loop so the DMA engines spend
            # the head exclusively on x1/wk1/wq1/wv1. (Each queue serializes
            # its transfers at ~1.7us fixed + bytes at ~140GB/s, and the
            # engine pool is shared across queues.)
            nc.sync.dma_start(x1ab[0][:], x1a_d[:])
            nc.scalar.dma_start(x1ab[1][:], x1b_d[:])
            _wv_dma(nc.gpsimd, 1)
            nc.gpsimd.dma_start(m4_sb[1][:].rearrange("p a b -> p (a b)"),
                                m4_d[1][:])
            # exp-table preload + ACT warm-up while the inputs stream in
            nc.scalar.activation(junk_e[:], junk[:], EXP)
            nc.gpsimd.memset(kz[1][:], 0.0)

            # The scheduler front-loads dependency-free DMA launches, so the
            # non-critical transfers would otherwise compete with x1 for the
            # shared DMA engines. Gate each one on x1's arrival by writing a
            # corner of its destination (WAW dep) sourced from x1b.
            gq = x1ab[1][0:1, 0:1]
            for corner in (
                x_r[2][0:1, 0, 0:1], x_r[2][0:1, 2, 0:1],
                w_r["wk1"][0:1, 1, 0, 0:1], w_r["wq1"][0:1, 1, 0, 0:1],
                w_r["wv2"][0:1, 0, 0:1],
                w_r["wq2"][0:1, 0, 0, 0:1], w_r["wk2"][0:1, 0, 0, 0:1],
            ):
                nc.gpsimd.tensor_copy(corner, gq)
            _w_dma(nc.gpsimd, "wk1", 1)
            _w_dma(nc.gpsimd, "wq1", 1)
            _x_dma(nc.sync, 2, 0)
            _x_dma(nc.sync, 2, 1)
            _wv_dma(nc.sync, 2)
            _w_dma_all(nc.sync, "wq2")
            _w_dma_all(nc.sync, "wk2")
            nc.gpsimd.dma_start(m4_sb[2][:].rearrange("p a b -> p (a b)"),
                                m4_d[2][:])
            nc.gpsimd.memset(kz[2][:], 0.0)

            # PE warm-up: the HAM clock gate releases only after ~3.4us of
            # sustained activity; these bridge the DMA window so the first
            # projections run at full rate.
            for _ in range(8):
                wps = _tt(stp, [128, 512], F32, "st")
                nc.tensor.matmul(wps[:], junk[:, 0:128], junk[:], start=True,
                                 stop=True)

            # ---- projection pieces ([128, 512] PSUM chunks) ----
            # qT per side: [128, 2, L] (tile ht holds heads 2ht, 2ht+1).
            # kz per side: [128, HPG*KT*128] zero-padded per (head, kt) block
            # so QK's moving qT streams all 128 partitions at full rate.
            def x_chunk(s, dk, c0, c1):
                # side 1's x lives in the packed x1a/x1b tiles
                if s == 1:
                    t, loc = x1ab[dk // 2], dk % 2
                    return t[:, loc * L + c0:loc * L + c1]
                return x_r[s][:, dk, c0:c1]

            def w_chunk(kind, s, ot, dk):
                if s == 1 and ot == 0:
                    t = x1ab[0] if kind == "q" else x1ab[1]
                    return t[:, 2 * L + dk * 128:2 * L + (dk + 1) * 128]
                return w_r[f"w{kind}{s}"][:, ot, dk, :]

            def k_piece(s, ot, nh, act_h0=False):
                # output chans = heads 2ot,2ot+1 ; keys nh*512:(nh+1)*512
                ps = _tt(stp, [128, 512], F32, "st")
                for dk in range(DT):
                    nc.tensor.matmul(
                        ps[:], w_chunk("k", s, ot, dk),
                        x_chunk(s, dk, nh * 512, (nh + 1) * 512),
                        start=(dk == 0), stop=(dk == DT - 1))
                for hh in range(2):
                    h = 2 * ot + hh
                    po = hh * 64
                    base = h * KT * 128 + nh * 512
                    # the pre-stream head-0 cast can ride the idle ACT engine
                    # to shorten the DVE chain ahead of the first QK
                    eng = nc.scalar.copy if (act_h0 and hh == 0) else \
                        nc.vector.tensor_copy
                    eng(kz[s][po:po + 64, base:base + 512], ps[po:po + 64, :])

            def q_piece(s, ot, nh, act=False):
                ps = _tt(stp, [128, 512], F32, "st")
                for dk in range(DT):
                    nc.tensor.matmul(
                        ps[:], w_chunk("q", s, ot, dk),
                        x_chunk(s, dk, nh * 512, (nh + 1) * 512),
                        start=(dk == 0), stop=(dk == DT - 1))
                eng = nc.scalar.copy if act else nc.vector.tensor_copy
                eng(qT[s][:, ot, nh * 512:(nh + 1) * 512], ps[:])

            # v in natural layout with mask column: [128, HPG, 65] per key tile
            def v_piece(s, lt):
                w = w_r[f"wv{s}"]
                ps = _tt(stp, [128, OG], F32, "st")
                for dk in range(DT):
                    nc.tensor.matmul(
                        ps[:], x_chunk(s, dk, lt * 128, (lt + 1) * 128),
                        w[:, dk, :], start=(dk == 0), stop=(dk == DT - 1))
                t = _tt(pers, [128, HPG, HD + 1], BF16, f"v{s}_{lt}")
                nc.vector.tensor_copy(
                    t[:, :, 0:HD], ps[:].rearrange("p (h d) -> p h d", h=HPG))
                # mask column on gpsimd: these tiny copies would head-of-line
                # block the critical kz/qT casts on the in-order DVE queue.
                nc.gpsimd.tensor_copy(t[:, :, HD:HD + 1],
                                      m4_sb[s][:, lt, :, None])
                assert len(v_e[s]) == lt
                v_e[s].append(t)

            # ---- prefix: just enough projection for branch 0's first QK ----
            # q first: the first QK needs both qT casts but only the first
            # kz cast, so this ordering minimizes the DVE chain ahead of it.
            q_piece(1, 0, 0)
            q_piece(1, 0, 1, act=True)
            k_piece(1, 0, 0, act_h0=True)
            k_piece(1, 0, 1)

            # Remaining pieces, spliced into branches in pairs (pairs keep
            # the 2-slot "st" rotation parity so QK keeps its one-tile
            # lookahead ahead of the exp stream). Branch 0 takes all of v1
            # at kt 1/3/5/7; later branches take pairs at kt 2/4/6.
            splices = {
                0: [[lambda: v_piece(1, 0), lambda: v_piece(1, 1)],
                    [lambda: v_piece(1, 2), lambda: v_piece(1, 3)],
                    [lambda: v_piece(1, 4), lambda: v_piece(1, 5)],
                    [lambda: v_piece(1, 6), lambda: v_piece(1, 7)]],
                1: [[lambda: k_piece(1, 1, 0), lambda: k_piece(1, 1, 1)],
                    [lambda: q_piece(1, 1, 0), lambda: q_piece(1, 1, 1)]],
                2: [[lambda: q_piece(2, 0, 0), lambda: q_piece(2, 0, 1)],
                    [lambda: q_piece(2, 1, 0), lambda: q_piece(2, 1, 1)]],
                3: [[lambda: k_piece(2, 0, 0), lambda: k_piece(2, 0, 1)],
                    [lambda: k_piece(2, 1, 0), lambda: k_piece(2, 1, 1)]],
                4: [[lambda: v_piece(2, 0), lambda: v_piece(2, 1)],
                    [lambda: v_piece(2, 2), lambda: v_piece(2, 3)]],
                5: [[lambda: v_piece(2, 4), lambda: v_piece(2, 5)],
                    [lambda: v_piece(2, 6), lambda: v_piece(2, 7)]],
            }

            # ---- attention ----
            branches = [(ks, qs, h) for ks in (1, 2) for qs in (1, 2)
                        for h in range(HPG)]

            def emit_qk(ks, qs, h, kt, stage=False):
                st = _tt(stp, [128, L], F32, "st")
                blk = (h * KT + kt) * 128
                for nh in range(2):
                    nc.tensor.matmul(
                        st[:, nh * 512:(nh + 1) * 512],
                        kz[ks][:, blk:blk + 128],
                        qT[qs][:, h // 2, nh * 512:(nh + 1) * 512],
                        start=True,
                        stop=True,
                    )
                if stage:
                    # bounce the logits to SBUF: the exp reads SBUF at the
                    # same ACT cost, but the PSUM st slot frees as soon as
                    # the copy lands — deepening the QK lookahead beyond the
                    # two PSUM slots so projection-splice bursts on the PE
                    # don't starve the exp stream.
                    ss = smp.tile([128, L], F32, tag="sst", name="sst", bufs=3)
                    nc.vector.tensor_copy(ss[:], st[:])
                    return ss
                return st

            def emit_combine(p):
                # two branches deferred: by now the gpsimd broadcast of r is
                # long done, so these DVE ops never block the pipeline.
                ks, qs, h, acc, rbc = p
                oslice = outacc[qs][:, h, :]
                if ks == 1:
                    nc.vector.tensor_mul(oslice, acc[0:HD, :], rbc[:])
                else:
                    tmp = _tt(smp, [64, L], BF16, "tmp")
                    nc.vector.tensor_mul(tmp[:], acc[0:HD, :], rbc[:])
                    nc.vector.tensor_add(oslice, oslice, tmp[:])
                    nc.sync.dma_start(
                        out_d[qs][h * HD:(h + 1) * HD, :], oslice)

            def norm_pend(ks, qs, h, acc):
                # normalization scalar r = 1/denom in [1, L] (query mask and
                # the 0.5 averaging factor are applied on the host),
                # broadcast to 64 partitions on the (otherwise idle) gpsimd.
                s_sb = _tt(smp, [1, L], F32, "s_sb")
                nc.vector.tensor_copy(s_sb[:], acc[HD:HD + 1, :])
                rinv = _tt(smp, [1, L], F32, "rinv")
                nc.vector.reciprocal_approx_fast(rinv[:], s_sb[:])
                rbc = _tt(smp, [64, L], F32, "rbc")
                nc.gpsimd.partition_broadcast(rbc[:], rinv[:])
                return (ks, qs, h, acc, rbc)

            norm_q = []  # accs awaiting normalize (one branch old)
            combine_q = []  # pends awaiting combine (two branches old)
            look_st = None
            for bi, (ks, qs, h) in enumerate(branches):
                last = bi == len(branches) - 1
                spl = splices.get(bi, [])
                # QK for all 8 key tiles first: the PE free-runs one tile
                # ahead of ACT (throttled by the two st PSUM slots).
                sts = [look_st] if look_st is not None else []
                kts = (1, 3, 5, 7) if bi == 0 else (2, 4, 6)
                for kt in range(len(sts), KT):
                    sts.append(emit_qk(ks, qs, h, kt))
                    if kt in kts and spl:
                        for fn in spl.pop(0):
                            fn()
                pts = []
                for kt in range(KT):
                    if (last and kt == KT - 1) or (bi == 0 and kt == 0):
                        # split tiles: the kernel's very first exp can start
                        # on the first QK half (one cast earlier), and the
                        # final exp's first half overlaps the raw export.
                        pair = []
                        for nhh in range(2):
                            pt = _tt(ptp, [128, 512], BF16, "pt")
                            nc.scalar.activation(
                                pt[:], sts[kt][:, nhh * 512:(nhh + 1) * 512],
                                EXP)
                            pair.append(pt)
                        pts.append(tuple(pair))
                    else:
                        pt = _tt(ptp, [128, L], BF16, "pt")
                        nc.scalar.activation(pt[:], sts[kt][:], EXP)
                        pts.append(pt)
                # flush: combines of branch i-2, then normalizes of branch
                # i-1 — everything data-ready by now, so the in-order DVE
                # queue never parks on them.
                while combine_q:
                    emit_combine(combine_q.pop(0))
                while norm_q:
                    combine_q.append(norm_pend(*norm_q.pop(0)))
                if bi == 14:
                    # branches 14/15 skip the on-device combine: ship their
                    # ks=1 partners' halves now, raw accumulators later.
                    for hh in (2, 3):
                        nc.sync.dma_start(
                            out_d[2][hh * HD:(hh + 1) * HD, :],
                            outacc[2][:, hh, :])
                acc = _tt(accp, [HD + 1, L], F32, "acc")
                for kt in range(KT):
                    if kt == KT - 1 and not last:
                        # software-pipeline: next branch's first QK goes ahead
                        # of this branch's last PV so ACT rolls over gap-free.
                        look_st = emit_qk(*branches[bi + 1][:3], 0)
                    if last and kt == KT - 1:
                        for nh in range(2):
                            nc.tensor.matmul(
                                acc[:, nh * 512:(nh + 1) * 512],
                                v_e[ks][kt][:, h, :], pts[KT - 1][nh][:],
                                start=False, stop=True)
                        for nh in range(2):
                            # raw export per column half: one DVE copy + DMA,
                            # no normalize chain in the kernel tail. The two
                            # halves ship on different queues.
                            sl = slice(nh * 512, (nh + 1) * 512)
                            o = _tt(smp, [HD + 1, L], F32, "oacc")
                            nc.vector.tensor_copy(o[:, sl], acc[:, sl])
                            eng = nc.sync if nh == 0 else nc.gpsimd
                            eng.dma_start(oa_d[15][:, sl], o[:, sl])
                    else:
                        for nh in range(2):
                            p = pts[kt]
                            rhs = (p[nh][:] if isinstance(p, tuple)
                                   else p[:, nh * 512:(nh + 1) * 512])
                            nc.tensor.matmul(
                                acc[:, nh * 512:(nh + 1) * 512],
                                v_e[ks][kt][:, h, :],
                                rhs,
                                start=(kt == 0),
                                stop=(kt == KT - 1),
                            )
                if bi == 14:
                    o = _tt(smp, [HD + 1, L], F32, "oacc")
                    nc.vector.tensor_copy(o[:], acc[:])
                    nc.sync.dma_start(oa_d[14][:], o[:])
                elif not last:
                    norm_q.append((ks, qs, h, acc))

    nc.compile()
    return nc


def kernel(**inputs):
    global _NC
    if _NC is None:
        _NC = _build()

    mask1 = np.asarray(inputs["mask1"], dtype=np.float32)
    mask2 = np.asarray(inputs["mask2"], dtype=np.float32)
    # pre-zero masked tokens: masked keys then contribute exp(0)*0 = 0 to
    # both the attention numerator and (via the v mask column) denominator.
    x1 = np.asarray(inputs["input1"], dtype=np.float32) * mask1[:, :, None]
    x2 = np.asarray(inputs["input2"], dtype=np.float32) * mask2[:, :, None]
    W = {k: np.asarray(inputs[k], dtype=np.float32) for k in
         ("Wq1", "Wk1", "Wv1", "Wq2", "Wk2", "Wv2")}

    def pack_x(xb):
        # [L, D] -> xT [D, L] -> [p][dk][c] flattened
        return np.ascontiguousarray(
            xb.T.astype(np.float16).reshape(DT, 128, L)
            .transpose(1, 0, 2).reshape(128, DT * L))

    def pack_wqk(w, og):
        # W.T[:, og] is [D, OG] = [dk p][ot c128] -> [p][ot][dk][c]
        return np.ascontiguousarray(
            w.T[:, og].astype(np.float16).reshape(DT, 128, 2, 128)
            .transpose(1, 2, 0, 3).reshape(128, 2 * DT * 128))

    def pack_wv(w, og):
        # [dk p][c] -> [p][dk][c]
        return np.ascontiguousarray(
            w.T[:, og].astype(np.float16).reshape(DT, 128, OG)
            .transpose(1, 0, 2).reshape(128, DT * OG))

    in_maps = []
    for core in range(8):
        b, hg = core // 2, core % 2
        og = slice(hg * OG, (hg + 1) * OG)
        xp1 = pack_x(x1[b])
        wq1p = pack_wqk(W["Wq1"], og)
        wk1p = pack_wqk(W["Wk1"], og)
        m = {
            "x1a": np.ascontiguousarray(
                np.concatenate([xp1[:, :2 * L], wq1p[:, :DT * 128]], axis=1)),
            "x1b": np.ascontiguousarray(
                np.concatenate([xp1[:, 2 * L:], wk1p[:, :DT * 128]], axis=1)),
            "x2T": pack_x(x2[b]),
            "m41": np.ascontiguousarray(
                np.repeat(mask1[b].reshape(KT, 128).T[:, :, None], HPG, axis=2)
                .reshape(128, KT * HPG)),
            "m42": np.ascontiguousarray(
                np.repeat(mask2[b].reshape(KT, 128).T[:, :, None], HPG, axis=2)
                .reshape(128, KT * HPG)),
        }
        for s in (1, 2):
            m[f"wq{s}"] = pack_wqk(W[f"Wq{s}"], og)
            m[f"wk{s}"] = pack_wqk(W[f"Wk{s}"], og)
            m[f"wv{s}"] = pack_wv(W[f"Wv{s}"], og)
        in_maps.append(m)

    global LAST_RESULT
    if TRACE:
        _install_ntff_hook()
    res = run_bass_kernel_spmd(_NC, in_maps, list(range(8)), trace=TRACE)
    LAST_RESULT = res

    # query-side mask + the 0.5 branch-average factor are applied here (the
    # device ships out = sum_branches O/denom, unmasked).
    hm1 = (0.5 * mask1)[:, :, None]
    hm2 = (0.5 * mask2)[:, :, None]
    output1 = np.empty((NB, L, D), dtype=np.float32)
    output2 = np.empty((NB, L, D), dtype=np.float32)
    for core in range(8):
        b, hg = core // 2, core % 2
        og = slice(hg * OG, (hg + 1) * OG)
        output1[b, :, og] = np.asarray(res.results[core]["out1T"],
                                       dtype=np.float32).T
        o2 = np.asarray(res.results[core]["out2T"], dtype=np.float32)
        # branches 14/15 = (ks=2, qs=2, h=2/3) shipped raw accumulators;
        # divide and add their terms to the ks=1 halves in out2T.
        for bi, hh in ((14, 2), (15, 3)):
            a = np.asarray(res.results[core][f"oa{bi}"], dtype=np.float32)
            o2[hh * HD:(hh + 1) * HD, :] += a[:HD, :] / a[HD:HD + 1, :]
        output2[b, :, og] = o2.T
    output1 *= hm1
    output2 *= hm2
    return (output1, output2)
